# revision 25
# baseline (speedup 1.0000x reference)
"""GQA attention kernel for 8 TRN2 NeuronCores.

Problem: B=2, T=2048, D=2048, H=16 q-heads, KV=4 kv-heads, HD=128, RoPE,
non-causal softmax, out projection. f32 reference.

Sharding: 8 cores = 2 batches x 4 kv-groups. Core c handles batch c//4 and
kv-group c%4 (4 q heads + 1 kv head). Each core computes a partial output
x @ wq_g -> attention -> (heads g) @ wo_g^T: full [T, D] partial summed on
host over the 4 groups of each batch (tensor-parallel unshard).

On-device layout: everything transposed ([hd, t], hd=128=partition dim).
 - host feeds xT, wqT, wkT, wvT (d-on-partition chunks) so projections are
   plain lhsT.T @ rhs matmuls with K=d contraction, fp32r (full PE rate).
 - scores computed transposed: ST[s, t] = k^T q per s-chunk; softmax over s
   (partitions) uses exp on ACT + bf16 chunk-adds on DVE + a ones-matmul
   partition-reduce-broadcast on PE; normalization folded into the OT evac.
 - PV: OT[hd, t] += v_nat[s, hd]^T expST[s, t] per s-chunk (bf16).
 - out projection: out[t, d] = sum_h OTn_h[j, t]^T wogT[j, d] (bf16).
"""
import os
import sys

for _p in ("/opt/trn_rl_repo", "/root/.axon_site/_ro/trn_rl_repo"):
    if os.path.isdir(_p) and _p not in sys.path:
        sys.path.append(_p)

import numpy as np
import ml_dtypes

import concourse.bass as bass
import concourse.tile as tile
from concourse.tile import add_dep_helper
from concourse import bacc, mybir
from concourse import bass_utils
from concourse.bass_utils import run_bass_kernel_spmd

# If a caller enables tracing (BASS_TRACE=1), artifact upload may have no
# bucket access in this container; fall back to the local dir.
_orig_upload = bass_utils.upload_artifacts


def _safe_upload(tmpdir):
    try:
        return _orig_upload(tmpdir)
    except Exception:
        return tmpdir


bass_utils.upload_artifacts = _safe_upload

B, T, D = 2, 2048, 2048
H, KV, HD = 16, 4, 128
NR = H // KV  # 4 q heads per kv group
NCORES = 8
ROPE_BASE = 10000.0
SCALE = float(HD) ** -0.5

F32R = mybir.dt.float32r
F32 = mybir.dt.float32
BF16 = mybir.dt.bfloat16

_cache = {}


def _build_nc():
    nc = bacc.Bacc("TRN2", target_bir_lowering=False, debug=False,
                   num_devices=NCORES)

    xt_e = nc.dram_tensor("xt", [128, 16, T], F32R, kind="ExternalInput").ap()
    wqt_e = [nc.dram_tensor(f"wqt{j}", [128, 16, HD], F32R,
                            kind="ExternalInput").ap() for j in range(NR)]
    wkt_e = nc.dram_tensor("wkt", [128, 16, HD], F32R, kind="ExternalInput").ap()
    wvt_e = nc.dram_tensor("wvt", [128, 16, HD], F32R, kind="ExternalInput").ap()
    wot_e = nc.dram_tensor("wot", [128, NR, D], BF16, kind="ExternalInput").ap()
    cos_e = nc.dram_tensor("cosa", [128, T], F32R, kind="ExternalInput").ap()
    sin_e = nc.dram_tensor("sina", [128, T], F32R, kind="ExternalInput").ap()
    ident_e = nc.dram_tensor("ident", [128, 128], BF16, kind="ExternalInput").ap()
    ones_e = nc.dram_tensor("ones", [128, 128], BF16, kind="ExternalInput").ap()
    out_e = nc.dram_tensor("out", [T, D], BF16, kind="ExternalOutput").ap()

    with tile.TileContext(nc) as tc:
        import contextlib
        with contextlib.ExitStack() as ctx:
            consts = ctx.enter_context(tc.tile_pool(name="consts", bufs=1))
            weights = ctx.enter_context(tc.tile_pool(name="weights", bufs=1))
            acts = ctx.enter_context(tc.tile_pool(name="acts", bufs=1))

            cos_sb = consts.tile([128, T], F32R, tag="cos")
            sin_sb = consts.tile([128, T], F32R, tag="sin")
            ident_sb = consts.tile([128, 128], BF16, tag="ident")
            ones_sb = consts.tile([128, 128], BF16, tag="ones")
            wkt_sb = weights.tile([128, 16, HD], F32R, tag="wkt")
            wvt_sb = weights.tile([128, 16, HD], F32R, tag="wvt")
            wqt_sb = [weights.tile([128, 16, HD], F32R, tag=f"wqt{j}",
                                   name=f"wqt{j}_sb") for j in range(NR)]
            wot_sb = weights.tile([128, NR, D], BF16, tag="wot")
            # DMA ordering: weights ride the FRONT of both HWDGE queues (all
            # of them land by ~12us) so the chunk-paced projection matmuls
            # never stall on a weight; xt tile halves stripe across both
            # queues behind them in consumption order. No artificial gating
            # (the queues drain in emission order anyway); only wot (needed
            # ~100us in) rides SWDGE gated behind tile-0 k-proj.
            nc.sync.dma_start(out=wkt_sb[:, :8, :], in_=wkt_e[:, :8, :])
            nc.sync.dma_start(out=wkt_sb[:, 8:, :], in_=wkt_e[:, 8:, :])
            nc.sync.dma_start(out=wqt_sb[0], in_=wqt_e[0])
            nc.scalar.dma_start(out=wvt_sb, in_=wvt_e)
            nc.scalar.dma_start(out=wqt_sb[2], in_=wqt_e[2])
            nc.scalar.dma_start(out=wqt_sb[3], in_=wqt_e[3])

            qtr = [acts.tile([128, T], F32R, tag=f"qtr{j}", name=f"qtr{j}") for j in range(NR)]
            ktr = acts.tile([128, T], F32R, tag="ktr")
            v_sb = acts.tile([128, 16, HD], BF16, tag="vsb")  # v natural, s-chunked

            # ---------------- Phase 1: projections + RoPE + v transpose ----
            with tc.tile_pool(name="xt", bufs=8) as xt_pool, \
                 tc.tile_pool(name="rope", bufs=2) as rope_pool, \
                 tc.tile_pool(name="p1ps", bufs=1, space="PSUM") as p1ps, \
                 tc.tile_pool(name="rotps", bufs=2, space="PSUM") as rotps:
                anchors = []  # tt0 k-proj matmul instructions
                for tt in range(4):
                    tsl = slice(tt * 512, (tt + 1) * 512)
                    xq = []
                    for i in range(4):
                        xti = xt_pool.tile([128, 4, 512], F32R, tag="xt")
                        lo = i * 4
                        nc.sync.dma_start(out=xti[:, 0:2, :],
                                          in_=xt_e[:, lo:lo + 2, tsl])
                        xq.append(xti)
                    if tt == 0:
                        # rope tables for t-tiles 0..1 behind xt0's sync half
                        nc.sync.dma_start(out=cos_sb[:, :1024],
                                          in_=cos_e[:, :1024])
                        nc.sync.dma_start(out=sin_sb[:, :1024],
                                          in_=sin_e[:, :1024])
                        nc.sync.dma_start(out=wqt_sb[1], in_=wqt_e[1])
                    for i in range(4):
                        lo = i * 4
                        nc.scalar.dma_start(out=xq[i][:, 2:4, :],
                                            in_=xt_e[:, lo + 2:lo + 4, tsl])
                    if tt == 0:
                        nc.scalar.dma_start(out=ident_sb, in_=ident_e)
                        nc.scalar.dma_start(out=ones_sb, in_=ones_e)
                    if tt == 1:
                        # rope tables for t-tiles 2..3; behind xt1, ahead of
                        # xt2/xt3 on each queue
                        nc.sync.dma_start(out=cos_sb[:, 1024:],
                                          in_=cos_e[:, 1024:])
                        nc.scalar.dma_start(out=sin_sb[:, 1024:],
                                            in_=sin_e[:, 1024:])
                    qps = [p1ps.tile([128, 512], F32, tag=f"qps{j}", name=f"qps{j}_{tt}") for j in range(NR)]
                    kps = p1ps.tile([128, 512], F32, tag="kps")
                    vps = p1ps.tile([128, 512], F32, tag="vps")
                    # chunk-paced emission in DMA-arrival order: [0:2] halves
                    # of each i-group (sync queue), then [2:4] (scalar queue);
                    # per chunk the projections run back-to-back so the PE
                    # stream tracks delivery with no FIFO head-blocking. For
                    # tile 0 only wkt/wqt0/wvt/wqt2/wqt3 ride ahead of x, so
                    # q1..q3 go in a second pass (their weights land while
                    # pass 1 runs).
                    order = [(i, dc) for dc2 in range(2) for i in range(4)
                             for dc in (2 * dc2, 2 * dc2 + 1)]
                    passes = ([[("k",), ("q", 0), ("v",)], [("q", 2), ("q", 3), ("q", 1)]]
                              if tt == 0 else
                              [[("k",), ("q", 0), ("q", 1), ("q", 2), ("q", 3), ("v",)]])
                    for ops in passes:
                        for pos, (i, dc) in enumerate(order):
                            g = i * 4 + dc
                            first, last = pos == 0, pos == 15
                            for op in ops:
                                if op[0] == "k":
                                    mk = nc.tensor.matmul(
                                        kps, wkt_sb[:, g, :], xq[i][:, dc, :],
                                        start=first, stop=last)
                                    if tt == 0:
                                        anchors.append(mk.ins)
                                elif op[0] == "v":
                                    nc.tensor.matmul(
                                        vps, wvt_sb[:, g, :], xq[i][:, dc, :],
                                        start=first, stop=last)
                                else:
                                    j = op[1]
                                    nc.tensor.matmul(
                                        qps[j], wqt_sb[j][:, g, :],
                                        xq[i][:, dc, :], start=first, stop=last)

                    # RoPE: dst = src*cos + rotate_half(src)*sin on DVE via
                    # partition-shifted PSUM reads (sign of the lower half
                    # folded into the host sin table). Mixed base partitions
                    # are only legal with a PSUM input, so the direct form
                    # must read PSUM — fine for tiles 0..2 where the banks
                    # recycle with slack.
                    def rope(src, dst):
                        t1 = rope_pool.tile([128, 512], F32R, tag="t1", name="t1")
                        nc.vector.tensor_mul(t1, src, cos_sb[:, tsl])
                        t2 = rope_pool.tile([128, 512], F32R, tag="t2", name="t2")
                        nc.vector.tensor_mul(t2[0:64, :], src[64:128, :],
                                             sin_sb[0:64, tsl])
                        nc.vector.tensor_mul(t2[64:128, :], src[0:64, :],
                                             sin_sb[64:128, tsl])
                        nc.gpsimd.tensor_add(dst, t1, t2)

                    # Tile 3's ropes gate phase 2: the score PSUM tiles reuse
                    # these banks, so holding them through a ~9us DVE chain
                    # stalls the PE. Instead evacuate each bank with one ACT
                    # copy (~0.6us), do the half-rotation with two SBUF->SBUF
                    # partition-shifted DMAs (queues are idle by now), and
                    # run base-aligned muls from SBUF.
                    def rope_staged(src, dst, name):
                        stg = rope_pool.tile([128, 512], F32R, tag="rstg",
                                             name=f"rstg_{name}")
                        nc.scalar.copy(stg, src)
                        stgr = rope_pool.tile([128, 512], F32R, tag="rstgr",
                                              name=f"rstgr_{name}")
                        nc.sync.dma_start(out=stgr[0:64, :], in_=stg[64:128, :])
                        nc.scalar.dma_start(out=stgr[64:128, :], in_=stg[0:64, :])
                        t1 = rope_pool.tile([128, 512], F32R, tag="t1", name="t1")
                        nc.vector.tensor_mul(t1, stg, cos_sb[:, tsl])
                        t2 = rope_pool.tile([128, 512], F32R, tag="t2", name="t2")
                        nc.vector.tensor_mul(t2, stgr, sin_sb[:, tsl])
                        nc.gpsimd.tensor_add(dst, t1, t2)

                    # v: copy vT psum -> sbuf bf16, PE-transpose 128-blocks
                    vt_sb = rope_pool.tile([128, 512], BF16, tag="vt")
                    nc.scalar.copy(vt_sb, vps)
                    if tt < 3:
                        rope(kps, ktr[:, tsl])
                    else:
                        rope_staged(kps, ktr[:, tsl], "k3")
                    for vb in range(4):
                        tr_ps = rotps.tile([128, 128], BF16, tag="rot")
                        nc.tensor.transpose(tr_ps, vt_sb[:, vb * 128:(vb + 1) * 128],
                                            ident_sb)
                        nc.vector.tensor_copy(v_sb[:, tt * 4 + vb, :], tr_ps)
                    for j in range(NR):
                        if tt < 3:
                            rope(qps[j], qtr[j][:, tsl])
                        else:
                            rope_staged(qps[j], qtr[j][:, tsl], f"q3_{j}")

            # ---------------- Phase 2+3: attention + out projection --------
            dwot = nc.gpsimd.dma_start(out=wot_sb, in_=wot_e)
            add_dep_helper(dwot.ins, anchors[15], reason="gate wot dma")
            with tc.tile_pool(name="p2sb", bufs=4) as p2sb, \
                 tc.tile_pool(name="dens", bufs=3) as dens, \
                 tc.tile_pool(name="otn", bufs=2) as otnp, \
                 tc.tile_pool(name="ostg", bufs=4) as ostg, \
                 tc.tile_pool(name="stps", bufs=2, space="PSUM") as stps, \
                 tc.tile_pool(name="otps", bufs=2, space="PSUM") as otps, \
                 tc.tile_pool(name="outps", bufs=2, space="PSUM") as outps:
                pending = [None]    # deferred softmax epilogue of previous head
                pend_out = []       # deferred out-projection pieces (prev tt)

                def flush_epilogue():
                    if pending[0] is not None:
                        pending[0]()
                        pending[0] = None

                def out_piece(tt, tkc, otn_t, dts):
                    # out-projection piece: 4 matmuls + evac + store per dt
                    rows = slice(tt * 512 + tkc * 128, tt * 512 + (tkc + 1) * 128)
                    for dt in dts:
                        o_ps = outps.tile([128, 512], F32, tag="ops",
                                          name=f"o_ps_{tt}_{tkc}_{dt}")
                        for hh in range(NR):
                            nc.tensor.matmul(
                                o_ps, otn_t[:, hh, tkc * 128:(tkc + 1) * 128],
                                wot_sb[:, hh, dt * 512:(dt + 1) * 512],
                                start=(hh == 0), stop=(hh == NR - 1))
                        o_sb = ostg.tile([128, 512], BF16, tag="ostg",
                                         name=f"o_sb_{tt}_{tkc}_{dt}")
                        nc.vector.tensor_copy(o_sb, o_ps)
                        nc.sync.dma_start(
                            out=out_e[rows, dt * 512:(dt + 1) * 512], in_=o_sb)

                for tt in range(4):
                    tsl = slice(tt * 512, (tt + 1) * 512)
                    otn_t = otnp.tile([128, NR, 512], BF16, tag="otn")
                    for h in range(NR):
                        ot_ps = otps.tile([128, 512], F32, tag="ot",
                                          name=f"ot_{tt}_{h}")
                        den = dens.tile([128, 2, 512], BF16, tag="den",
                                        name=f"den_{tt}_{h}")
                        exs = {}
                        # one-deep software pipeline: ST(sg) runs one step
                        # ahead of PV(sg) so PE never waits on the exp
                        for step in range(9):
                            if step < 8:
                                st_ps = stps.tile([128, 2, 512], F32, tag="st",
                                                  name=f"st_{tt}_{h}_{step}")
                                for half in range(2):
                                    sc = step * 2 + half
                                    nc.tensor.matmul(
                                        st_ps[:, half, :],
                                        ktr[:, sc * 128:(sc + 1) * 128],
                                        qtr[h][:, tsl], start=True, stop=True)
                                ex = p2sb.tile([128, 2, 512], BF16, tag="exp",
                                               name=f"ex_{tt}_{h}_{step}")
                                nc.scalar.activation(
                                    ex, st_ps, mybir.ActivationFunctionType.Exp,
                                    scale=SCALE)
                                exs[step] = ex
                                if step == 0:
                                    nc.vector.tensor_copy(den, ex)
                                else:
                                    nc.vector.tensor_add(den, den, ex)
                            if step >= 1:
                                sg = step - 1
                                for half in range(2):
                                    sc = sg * 2 + half
                                    nc.tensor.matmul(ot_ps, v_sb[:, sc, :],
                                                     exs[sg][:, half, :],
                                                     start=(sc == 0),
                                                     stop=(sc == 15))
                                if sg > 1:
                                    exs.pop(sg - 2, None)
                            if step == 2:
                                # previous head's epilogue: overlaps this
                                # head's score stream
                                flush_epilogue()
                            if step in (2, 4, 6, 8) and pend_out:
                                # a piece of the previous t-tile's out
                                # projection as PE filler; spread so the PE
                                # stays ahead of the exp stream every 2 steps
                                pend_out.pop(0)()

                        def epilogue(ot_ps=ot_ps, den=den, h=h,
                                     otn_t=otn_t, tt=tt):
                            # partition-reduce+broadcast denominator on PE,
                            # both halves accumulated into one PSUM bank
                            bc_ps = outps.tile([128, 512], F32, tag="ops",
                                               name=f"bc_{tt}_{h}")
                            nc.tensor.matmul(bc_ps, ones_sb, den[:, 0, :],
                                             start=True, stop=False)
                            nc.tensor.matmul(bc_ps, ones_sb, den[:, 1, :],
                                             start=False, stop=True)
                            rden = dens.tile([128, 512], F32, tag="rden",
                                             name=f"rden_{tt}_{h}")
                            nc.vector.reciprocal_approx_fast(rden, bc_ps)
                            nc.vector.tensor_tensor(out=otn_t[:, h, :], in0=ot_ps,
                                                    in1=rden,
                                                    op=mybir.AluOpType.mult)
                        pending[0] = epilogue

                    flush_epilogue()
                    pend_out = [
                        (lambda tt=tt, tkc=tkc, otn_t=otn_t, dt=dt:
                         out_piece(tt, tkc, otn_t, (dt,)))
                        for tkc in range(4) for dt in range(4)]
                # final t-tile's out projection
                for p in pend_out:
                    p()
    nc.compile()
    return nc


def _get_nc():
    if "nc" not in _cache:
        _cache["nc"] = _build_nc()
    return _cache["nc"]


def _host_consts():
    if "consts" in _cache:
        return _cache["consts"]
    inv = 1.0 / (ROPE_BASE ** (np.arange(0, HD, 2, dtype=np.float64) / HD))
    freqs = np.outer(np.arange(T, dtype=np.float64), inv)  # [T, 64]
    emb = np.concatenate([freqs, freqs], axis=-1)  # [T, 128]
    cos_t = np.cos(emb).T.astype(np.float32).copy()  # [128, T]
    sin_t = np.sin(emb).T.astype(np.float32).copy()
    sin_t[:64, :] *= -1.0  # rotate-half sign folded in (see rope())
    ident = np.eye(128, dtype=ml_dtypes.bfloat16)
    ones = np.ones((128, 128), dtype=ml_dtypes.bfloat16)
    _cache["consts"] = (cos_t, sin_t, ident, ones)
    return _cache["consts"]


def _in_maps(x, wq, wk, wv, wo):
    cos_t, sin_t, ident, ones = _host_consts()
    maps = []
    for c in range(NCORES):
        b, g = c // KV, c % KV
        xt = np.ascontiguousarray(
            x[b].reshape(T, 16, 128).transpose(2, 1, 0)).astype(np.float32)
        wq_g = wq[g * NR * HD:(g + 1) * NR * HD]  # [512, D]
        # per-head contiguous slices: wqt{j}[p, dc, jc] = wq_g[j*128+jc, dc*128+p]
        wq_h = wq_g.reshape(NR, HD, 16, 128).transpose(0, 3, 2, 1)  # [j, p, dc, jc]
        wk_g = wk[g * HD:(g + 1) * HD]
        wkt = np.ascontiguousarray(wk_g.reshape(HD, 16, 128).transpose(2, 1, 0))
        wv_g = wv[g * HD:(g + 1) * HD]
        wvt = np.ascontiguousarray(wv_g.reshape(HD, 16, 128).transpose(2, 1, 0))
        wo_g = wo[:, g * NR * HD:(g + 1) * NR * HD]  # [D, 512]
        wot = np.ascontiguousarray(
            wo_g.reshape(D, NR, 128).transpose(2, 1, 0)).astype(ml_dtypes.bfloat16)
        m = {
            "xt": xt, "wkt": wkt.astype(np.float32),
            "wvt": wvt.astype(np.float32), "wot": wot,
            "cosa": cos_t, "sina": sin_t,
            "ident": ident, "ones": ones,
        }
        for j in range(NR):
            m[f"wqt{j}"] = np.ascontiguousarray(wq_h[j]).astype(np.float32)
        maps.append(m)
    return maps


def run_spmd(x, wq, wk, wv, wo, **kw):
    nc = _get_nc()
    maps = _in_maps(x, wq, wk, wv, wo)
    return run_bass_kernel_spmd(nc, maps, core_ids=list(range(NCORES)), **kw)


def kernel(x, wq, wk, wv, wo):
    res = run_spmd(x, wq, wk, wv, wo)
    out = np.zeros((B, T, D), dtype=np.float32)
    for c in range(NCORES):
        out[c // KV] += res.results[c]["out"].astype(np.float32)
    return out



# revision 32
# speedup vs baseline: 1.1406x; 1.1406x over previous
"""GQA attention kernel for 8 TRN2 NeuronCores.

Problem: B=2, T=2048, D=2048, H=16 q-heads, KV=4 kv-heads, HD=128, RoPE,
non-causal softmax, out projection. f32 reference.

Sharding: 8 cores = 2 batches x 4 kv-groups. Core c handles batch c//4 and
kv-group c%4 (4 q heads + 1 kv head). Each core computes a partial output
x @ wq_g -> attention -> (heads g) @ wo_g^T: full [T, D] partial summed on
host over the 4 groups of each batch (tensor-parallel unshard).

On-device layout: everything transposed ([hd, t], hd=128=partition dim).
 - host feeds xT, wqT, wkT, wvT (d-on-partition chunks) so projections are
   plain lhsT.T @ rhs matmuls with K=d contraction, fp32r (full PE rate).
 - scores computed transposed: ST[s, t] = k^T q per s-chunk; softmax over s
   (partitions) uses exp on ACT + bf16 chunk-adds on DVE + a ones-matmul
   partition-reduce-broadcast on PE; normalization folded into the OT evac.
 - PV: OT[hd, t] += v_nat[s, hd]^T expST[s, t] per s-chunk (bf16).
 - out projection: out[t, d] = sum_h OTn_h[j, t]^T wogT[j, d] (bf16).
"""
import os
import sys

for _p in ("/opt/trn_rl_repo", "/root/.axon_site/_ro/trn_rl_repo"):
    if os.path.isdir(_p) and _p not in sys.path:
        sys.path.append(_p)

import numpy as np
import ml_dtypes

import concourse.bass as bass
import concourse.tile as tile
from concourse.tile import add_dep_helper
from concourse import bacc, mybir
from concourse import bass_utils
from concourse.bass_utils import run_bass_kernel_spmd

# If a caller enables tracing (BASS_TRACE=1), artifact upload may have no
# bucket access in this container; fall back to the local dir.
_orig_upload = bass_utils.upload_artifacts


def _safe_upload(tmpdir):
    try:
        return _orig_upload(tmpdir)
    except Exception:
        return tmpdir


bass_utils.upload_artifacts = _safe_upload

B, T, D = 2, 2048, 2048
H, KV, HD = 16, 4, 128
NR = H // KV  # 4 q heads per kv group
NCORES = 8
ROPE_BASE = 10000.0
SCALE = float(HD) ** -0.5

F32R = mybir.dt.float32r
F32 = mybir.dt.float32
BF16 = mybir.dt.bfloat16

_cache = {}


def _build_nc():
    nc = bacc.Bacc("TRN2", target_bir_lowering=False, debug=False,
                   num_devices=NCORES)

    xt_e = nc.dram_tensor("xt", [128, 16, T], F32R, kind="ExternalInput").ap()
    wqt_e = [nc.dram_tensor(f"wqt{j}", [128, 16, HD], F32R,
                            kind="ExternalInput").ap() for j in range(NR)]
    wkt_e = nc.dram_tensor("wkt", [128, 16, HD], F32R, kind="ExternalInput").ap()
    wvt_e = nc.dram_tensor("wvt", [128, 16, HD], F32R, kind="ExternalInput").ap()
    wot_e = nc.dram_tensor("wot", [128, NR, D], BF16, kind="ExternalInput").ap()
    cos_e = nc.dram_tensor("cosa", [128, T], F32R, kind="ExternalInput").ap()
    sin_e = nc.dram_tensor("sina", [128, T], F32R, kind="ExternalInput").ap()
    ident_e = nc.dram_tensor("ident", [128, 128], BF16, kind="ExternalInput").ap()
    ones_e = nc.dram_tensor("ones", [128, 128], BF16, kind="ExternalInput").ap()
    out_e = nc.dram_tensor("out", [T, D], BF16, kind="ExternalOutput").ap()

    with tile.TileContext(nc) as tc:
        import contextlib
        with contextlib.ExitStack() as ctx:
            consts = ctx.enter_context(tc.tile_pool(name="consts", bufs=1))
            weights = ctx.enter_context(tc.tile_pool(name="weights", bufs=1))
            acts = ctx.enter_context(tc.tile_pool(name="acts", bufs=1))

            cos_sb = consts.tile([128, T], F32R, tag="cos")
            sin_sb = consts.tile([128, T], F32R, tag="sin")
            ident_sb = consts.tile([128, 128], BF16, tag="ident")
            ones_sb = consts.tile([128, 128], BF16, tag="ones")
            wkt_sb = weights.tile([128, 16, HD], F32R, tag="wkt")
            wvt_sb = weights.tile([128, 16, HD], F32R, tag="wvt")
            wqt_sb = [weights.tile([128, 16, HD], F32R, tag=f"wqt{j}",
                                   name=f"wqt{j}_sb") for j in range(NR)]
            wot_sb = weights.tile([128, NR, D], BF16, tag="wot")
            # DMA ordering: weights ride the FRONT of both HWDGE queues (all
            # of them land by ~12us) so the chunk-paced projection matmuls
            # never stall on a weight; xt tile halves stripe across both
            # queues behind them in consumption order. No artificial gating
            # (the queues drain in emission order anyway); only wot (needed
            # ~100us in) rides SWDGE gated behind tile-0 k-proj.
            nc.sync.dma_start(out=wkt_sb[:, :8, :], in_=wkt_e[:, :8, :])
            nc.sync.dma_start(out=wkt_sb[:, 8:, :], in_=wkt_e[:, 8:, :])
            nc.sync.dma_start(out=wqt_sb[0], in_=wqt_e[0])
            nc.scalar.dma_start(out=wvt_sb, in_=wvt_e)
            nc.scalar.dma_start(out=wqt_sb[2], in_=wqt_e[2])
            nc.scalar.dma_start(out=wqt_sb[3], in_=wqt_e[3])

            qtr = [acts.tile([128, T], F32R, tag=f"qtr{j}", name=f"qtr{j}") for j in range(NR)]
            ktr = acts.tile([128, T], F32R, tag="ktr")
            v_sb = acts.tile([128, 16, HD], BF16, tag="vsb")  # v natural, s-chunked

            # ---------------- Phase 1: projections + RoPE + v transpose ----
            with tc.tile_pool(name="xt", bufs=8) as xt_pool, \
                 tc.tile_pool(name="rope", bufs=2) as rope_pool, \
                 tc.tile_pool(name="stage", bufs=3) as stage_pool, \
                 tc.tile_pool(name="p1ps", bufs=1, space="PSUM") as p1ps, \
                 tc.tile_pool(name="rotps", bufs=2, space="PSUM") as rotps:
                anchors = []  # tt0 k-proj matmul instructions
                for tt in range(4):
                    tsl = slice(tt * 512, (tt + 1) * 512)
                    xq = []
                    for i in range(4):
                        xti = xt_pool.tile([128, 4, 512], F32R, tag="xt")
                        lo = i * 4
                        nc.sync.dma_start(out=xti[:, 0:2, :],
                                          in_=xt_e[:, lo:lo + 2, tsl])
                        xq.append(xti)
                    if tt == 0:
                        # cos for t-tiles 0..1 behind xt0's sync half; sin
                        # rides scalar ahead of xt0[2:4] (rope k0 needs both
                        # by ~30us); wqt1 after xt0 on scalar — pass 2 uses
                        # q1 last
                        nc.sync.dma_start(out=cos_sb[:, :1024],
                                          in_=cos_e[:, :1024])
                        nc.scalar.dma_start(out=sin_sb[:, :1024],
                                            in_=sin_e[:, :1024])
                    for i in range(4):
                        lo = i * 4
                        nc.scalar.dma_start(out=xq[i][:, 2:4, :],
                                            in_=xt_e[:, lo + 2:lo + 4, tsl])
                    if tt == 0:
                        nc.scalar.dma_start(out=wqt_sb[1], in_=wqt_e[1])
                        nc.scalar.dma_start(out=ident_sb, in_=ident_e)
                        nc.scalar.dma_start(out=ones_sb, in_=ones_e)
                    if tt == 1:
                        # rope tables for t-tiles 2..3; behind xt1, ahead of
                        # xt2/xt3 on each queue
                        nc.sync.dma_start(out=cos_sb[:, 1024:],
                                          in_=cos_e[:, 1024:])
                        nc.scalar.dma_start(out=sin_sb[:, 1024:],
                                            in_=sin_e[:, 1024:])
                    qps = [p1ps.tile([128, 512], F32, tag=f"qps{j}", name=f"qps{j}_{tt}") for j in range(NR)]
                    kps = p1ps.tile([128, 512], F32, tag="kps")
                    vps = p1ps.tile([128, 512], F32, tag="vps")
                    # chunk-paced emission in DMA-arrival order: [0:2] halves
                    # of each i-group (sync queue), then [2:4] (scalar queue);
                    # per chunk the projections run back-to-back so the PE
                    # stream tracks delivery with no FIFO head-blocking. For
                    # tile 0 only wkt/wqt0/wvt/wqt2/wqt3 ride ahead of x, so
                    # q1..q3 go in a second pass (their weights land while
                    # pass 1 runs).
                    order = [(i, dc) for dc2 in range(2) for i in range(4)
                             for dc in (2 * dc2, 2 * dc2 + 1)]
                    passes = ([[("k",), ("q", 0), ("v",)], [("q", 2), ("q", 3), ("q", 1)]]
                              if tt == 0 else
                              [[("k",), ("q", 0), ("q", 1), ("q", 2), ("q", 3), ("v",)]])
                    for ops in passes:
                        for pos, (i, dc) in enumerate(order):
                            g = i * 4 + dc
                            first, last = pos == 0, pos == 15
                            for op in ops:
                                if op[0] == "k":
                                    mk = nc.tensor.matmul(
                                        kps, wkt_sb[:, g, :], xq[i][:, dc, :],
                                        start=first, stop=last)
                                    if tt == 0:
                                        anchors.append(mk.ins)
                                elif op[0] == "v":
                                    nc.tensor.matmul(
                                        vps, wvt_sb[:, g, :], xq[i][:, dc, :],
                                        start=first, stop=last)
                                else:
                                    j = op[1]
                                    nc.tensor.matmul(
                                        qps[j], wqt_sb[j][:, g, :],
                                        xq[i][:, dc, :], start=first, stop=last)

                    # RoPE: dst = src*cos + rotate_half(src)*sin on DVE via
                    # partition-shifted PSUM reads (sign of the lower half
                    # folded into the host sin table). Mixed base partitions
                    # are only legal with a PSUM input, so the direct form
                    # must read PSUM — fine for tiles 0..2 where the banks
                    # recycle with slack.
                    def rope(src, dst):
                        t1 = rope_pool.tile([128, 512], F32R, tag="t1", name="t1")
                        nc.vector.tensor_mul(t1, src, cos_sb[:, tsl])
                        t2 = rope_pool.tile([128, 512], F32R, tag="t2", name="t2")
                        nc.vector.tensor_mul(t2[0:64, :], src[64:128, :],
                                             sin_sb[0:64, tsl])
                        nc.vector.tensor_mul(t2[64:128, :], src[0:64, :],
                                             sin_sb[64:128, tsl])
                        nc.gpsimd.tensor_add(dst, t1, t2)

                    # v: copy vT psum -> sbuf bf16, PE-transpose 128-blocks
                    vt_sb = rope_pool.tile([128, 512], BF16, tag="vt")
                    nc.scalar.copy(vt_sb, vps)
                    if tt < 3:
                        rope(kps, ktr[:, tsl])
                        for vb in range(4):
                            tr_ps = rotps.tile([128, 128], BF16, tag="rot")
                            nc.tensor.transpose(
                                tr_ps, vt_sb[:, vb * 128:(vb + 1) * 128],
                                ident_sb)
                            nc.vector.tensor_copy(v_sb[:, tt * 4 + vb, :], tr_ps)
                        for j in range(NR):
                            rope(qps[j], qtr[j][:, tsl])
                    else:
                        # Tile 3's ropes gate phase 2: the score PSUM tiles
                        # reuse these banks, so holding them through a ~9us
                        # DVE chain stalls the PE. Evacuate all five banks
                        # FIRST, split across the scalar+vector engines
                        # (~2.5us to free everything), then do the
                        # half-rotation with partition-shifted SBUF->SBUF
                        # DMAs (queues are idle by now) and base-aligned
                        # muls from SBUF.
                        # evacuate straight into the rope DESTINATION slice
                        # (scratch-free), rope in-place afterwards
                        srcs = [kps] + qps
                        dsts = [ktr[:, tsl]] + [qtr[j][:, tsl] for j in range(NR)]
                        for r, (src, dst) in enumerate(zip(srcs, dsts)):
                            if r % 2 == 0:
                                nc.scalar.copy(dst, src)
                            else:
                                nc.vector.tensor_copy(dst, src)
                        for vb in range(4):
                            tr_ps = rotps.tile([128, 128], BF16, tag="rot")
                            nc.tensor.transpose(
                                tr_ps, vt_sb[:, vb * 128:(vb + 1) * 128],
                                ident_sb)
                            nc.vector.tensor_copy(v_sb[:, tt * 4 + vb, :], tr_ps)
                        for stg, dst in zip(dsts, dsts):
                            stgr = stage_pool.tile([128, 512], F32R,
                                                   tag="rstgr", name="rstgr")
                            nc.sync.dma_start(out=stgr[0:64, :],
                                              in_=stg[64:128, :])
                            nc.scalar.dma_start(out=stgr[64:128, :],
                                                in_=stg[0:64, :])
                            t1 = rope_pool.tile([128, 512], F32R, tag="t1",
                                                name="t1")
                            nc.vector.tensor_mul(t1, stg, cos_sb[:, tsl])
                            t2 = rope_pool.tile([128, 512], F32R, tag="t2",
                                                name="t2")
                            nc.vector.tensor_mul(t2, stgr, sin_sb[:, tsl])
                            nc.gpsimd.tensor_add(dst, t1, t2)

            # ---------------- Phase 2+3: attention + out projection --------
            dwot = nc.gpsimd.dma_start(out=wot_sb, in_=wot_e)
            add_dep_helper(dwot.ins, anchors[15], reason="gate wot dma")
            with tc.tile_pool(name="p2sb", bufs=4) as p2sb, \
                 tc.tile_pool(name="dens", bufs=3) as dens, \
                 tc.tile_pool(name="otn", bufs=2) as otnp, \
                 tc.tile_pool(name="ostg", bufs=4) as ostg, \
                 tc.tile_pool(name="stps", bufs=2, space="PSUM") as stps, \
                 tc.tile_pool(name="otps", bufs=2, space="PSUM") as otps, \
                 tc.tile_pool(name="outps", bufs=2, space="PSUM") as outps:
                pending = [None]    # deferred softmax epilogue of previous head
                pend_out = []       # deferred out-projection pieces (prev tt)

                def flush_epilogue():
                    if pending[0] is not None:
                        pending[0]()
                        pending[0] = None

                def out_piece(tt, tkc, otn_t, dts):
                    # out-projection piece: 4 matmuls + evac + store per dt
                    rows = slice(tt * 512 + tkc * 128, tt * 512 + (tkc + 1) * 128)
                    for dt in dts:
                        o_ps = outps.tile([128, 512], F32, tag="ops",
                                          name=f"o_ps_{tt}_{tkc}_{dt}")
                        for hh in range(NR):
                            nc.tensor.matmul(
                                o_ps, otn_t[:, hh, tkc * 128:(tkc + 1) * 128],
                                wot_sb[:, hh, dt * 512:(dt + 1) * 512],
                                start=(hh == 0), stop=(hh == NR - 1))
                        o_sb = ostg.tile([128, 512], BF16, tag="ostg",
                                         name=f"o_sb_{tt}_{tkc}_{dt}")
                        nc.vector.tensor_copy(o_sb, o_ps)
                        nc.sync.dma_start(
                            out=out_e[rows, dt * 512:(dt + 1) * 512], in_=o_sb)

                for tt in range(4):
                    tsl = slice(tt * 512, (tt + 1) * 512)
                    otn_t = otnp.tile([128, NR, 512], BF16, tag="otn")
                    for h in range(NR):
                        ot_ps = otps.tile([128, 512], F32, tag="ot",
                                          name=f"ot_{tt}_{h}")
                        den = dens.tile([128, 2, 512], BF16, tag="den",
                                        name=f"den_{tt}_{h}")
                        exs = {}
                        # one-deep software pipeline: ST(sg) runs one step
                        # ahead of PV(sg) so PE never waits on the exp
                        for step in range(9):
                            if step < 8:
                                st_ps = stps.tile([128, 2, 512], F32, tag="st",
                                                  name=f"st_{tt}_{h}_{step}")
                                for half in range(2):
                                    sc = step * 2 + half
                                    nc.tensor.matmul(
                                        st_ps[:, half, :],
                                        ktr[:, sc * 128:(sc + 1) * 128],
                                        qtr[h][:, tsl], start=True, stop=True)
                                ex = p2sb.tile([128, 2, 512], BF16, tag="exp",
                                               name=f"ex_{tt}_{h}_{step}")
                                nc.scalar.activation(
                                    ex, st_ps, mybir.ActivationFunctionType.Exp,
                                    scale=SCALE)
                                exs[step] = ex
                                if step == 0:
                                    nc.vector.tensor_copy(den, ex)
                                else:
                                    nc.vector.tensor_add(den, den, ex)
                            if step >= 1:
                                sg = step - 1
                                for half in range(2):
                                    sc = sg * 2 + half
                                    nc.tensor.matmul(ot_ps, v_sb[:, sc, :],
                                                     exs[sg][:, half, :],
                                                     start=(sc == 0),
                                                     stop=(sc == 15))
                                if sg > 1:
                                    exs.pop(sg - 2, None)
                            if step == 2:
                                # previous head's epilogue: overlaps this
                                # head's score stream
                                flush_epilogue()
                            if step in (3, 5) and pend_out:
                                # a piece of the previous t-tile's out
                                # projection as PE filler
                                pend_out.pop(0)()

                        def epilogue(ot_ps=ot_ps, den=den, h=h,
                                     otn_t=otn_t, tt=tt):
                            # partition-reduce+broadcast denominator on PE,
                            # both halves accumulated into one PSUM bank
                            bc_ps = outps.tile([128, 512], F32, tag="ops",
                                               name=f"bc_{tt}_{h}")
                            nc.tensor.matmul(bc_ps, ones_sb, den[:, 0, :],
                                             start=True, stop=False)
                            nc.tensor.matmul(bc_ps, ones_sb, den[:, 1, :],
                                             start=False, stop=True)
                            rden = dens.tile([128, 512], F32, tag="rden",
                                             name=f"rden_{tt}_{h}")
                            nc.vector.reciprocal_approx_fast(rden, bc_ps)
                            nc.vector.tensor_tensor(out=otn_t[:, h, :], in0=ot_ps,
                                                    in1=rden,
                                                    op=mybir.AluOpType.mult)
                        pending[0] = epilogue

                    flush_epilogue()
                    pend_out = [
                        (lambda tt=tt, tkc=tkc, otn_t=otn_t, dts=dts:
                         out_piece(tt, tkc, otn_t, dts))
                        for tkc in range(4) for dts in ((0, 1), (2, 3))]
                # final t-tile's out projection
                for p in pend_out:
                    p()
    nc.compile()
    return nc


def _get_nc():
    if "nc" not in _cache:
        _cache["nc"] = _build_nc()
    return _cache["nc"]


def _host_consts():
    if "consts" in _cache:
        return _cache["consts"]
    inv = 1.0 / (ROPE_BASE ** (np.arange(0, HD, 2, dtype=np.float64) / HD))
    freqs = np.outer(np.arange(T, dtype=np.float64), inv)  # [T, 64]
    emb = np.concatenate([freqs, freqs], axis=-1)  # [T, 128]
    cos_t = np.cos(emb).T.astype(np.float32).copy()  # [128, T]
    sin_t = np.sin(emb).T.astype(np.float32).copy()
    sin_t[:64, :] *= -1.0  # rotate-half sign folded in (see rope())
    ident = np.eye(128, dtype=ml_dtypes.bfloat16)
    ones = np.ones((128, 128), dtype=ml_dtypes.bfloat16)
    _cache["consts"] = (cos_t, sin_t, ident, ones)
    return _cache["consts"]


def _in_maps(x, wq, wk, wv, wo):
    cos_t, sin_t, ident, ones = _host_consts()
    maps = []
    for c in range(NCORES):
        b, g = c // KV, c % KV
        xt = np.ascontiguousarray(
            x[b].reshape(T, 16, 128).transpose(2, 1, 0)).astype(np.float32)
        wq_g = wq[g * NR * HD:(g + 1) * NR * HD]  # [512, D]
        # per-head contiguous slices: wqt{j}[p, dc, jc] = wq_g[j*128+jc, dc*128+p]
        wq_h = wq_g.reshape(NR, HD, 16, 128).transpose(0, 3, 2, 1)  # [j, p, dc, jc]
        wk_g = wk[g * HD:(g + 1) * HD]
        wkt = np.ascontiguousarray(wk_g.reshape(HD, 16, 128).transpose(2, 1, 0))
        wv_g = wv[g * HD:(g + 1) * HD]
        wvt = np.ascontiguousarray(wv_g.reshape(HD, 16, 128).transpose(2, 1, 0))
        wo_g = wo[:, g * NR * HD:(g + 1) * NR * HD]  # [D, 512]
        wot = np.ascontiguousarray(
            wo_g.reshape(D, NR, 128).transpose(2, 1, 0)).astype(ml_dtypes.bfloat16)
        m = {
            "xt": xt, "wkt": wkt.astype(np.float32),
            "wvt": wvt.astype(np.float32), "wot": wot,
            "cosa": cos_t, "sina": sin_t,
            "ident": ident, "ones": ones,
        }
        for j in range(NR):
            m[f"wqt{j}"] = np.ascontiguousarray(wq_h[j]).astype(np.float32)
        maps.append(m)
    return maps


def run_spmd(x, wq, wk, wv, wo, **kw):
    nc = _get_nc()
    maps = _in_maps(x, wq, wk, wv, wo)
    return run_bass_kernel_spmd(nc, maps, core_ids=list(range(NCORES)), **kw)


def kernel(x, wq, wk, wv, wo):
    res = run_spmd(x, wq, wk, wv, wo)
    out = np.zeros((B, T, D), dtype=np.float32)
    for c in range(NCORES):
        out[c // KV] += res.results[c]["out"].astype(np.float32)
    return out



# revision 37
# speedup vs baseline: 1.1531x; 1.0109x over previous
"""GQA attention kernel for 8 TRN2 NeuronCores.

Problem: B=2, T=2048, D=2048, H=16 q-heads, KV=4 kv-heads, HD=128, RoPE,
non-causal softmax, out projection. f32 reference.

Sharding: 8 cores = 2 batches x 4 kv-groups. Core c handles batch c//4 and
kv-group c%4 (4 q heads + 1 kv head). Each core computes a partial output
x @ wq_g -> attention -> (heads g) @ wo_g^T: full [T, D] partial summed on
host over the 4 groups of each batch (tensor-parallel unshard).

On-device layout: everything transposed ([hd, t], hd=128=partition dim).
 - host feeds xT, wqT, wkT, wvT (d-on-partition chunks) so projections are
   plain lhsT.T @ rhs matmuls with K=d contraction, fp32r (full PE rate).
 - scores computed transposed: ST[s, t] = k^T q per s-chunk; softmax over s
   (partitions) uses exp on ACT + bf16 chunk-adds on DVE + a ones-matmul
   partition-reduce-broadcast on PE; normalization folded into the OT evac.
 - PV: OT[hd, t] += v_nat[s, hd]^T expST[s, t] per s-chunk (bf16).
 - out projection: out[t, d] = sum_h OTn_h[j, t]^T wogT[j, d] (bf16).
"""
import os
import sys

for _p in ("/opt/trn_rl_repo", "/root/.axon_site/_ro/trn_rl_repo"):
    if os.path.isdir(_p) and _p not in sys.path:
        sys.path.append(_p)

import numpy as np
import ml_dtypes

import concourse.bass as bass
import concourse.tile as tile
from concourse.tile import add_dep_helper
from concourse import bacc, mybir
from concourse import bass_utils
from concourse.bass_utils import run_bass_kernel_spmd

# If a caller enables tracing (BASS_TRACE=1), artifact upload may have no
# bucket access in this container; fall back to the local dir.
_orig_upload = bass_utils.upload_artifacts


def _safe_upload(tmpdir):
    try:
        return _orig_upload(tmpdir)
    except Exception:
        return tmpdir


bass_utils.upload_artifacts = _safe_upload

B, T, D = 2, 2048, 2048
H, KV, HD = 16, 4, 128
NR = H // KV  # 4 q heads per kv group
NCORES = 8
ROPE_BASE = 10000.0
SCALE = float(HD) ** -0.5

F32R = mybir.dt.float32r
F32 = mybir.dt.float32
BF16 = mybir.dt.bfloat16

_cache = {}


def _build_nc():
    nc = bacc.Bacc("TRN2", target_bir_lowering=False, debug=False,
                   num_devices=NCORES)

    xt_e = nc.dram_tensor("xt", [128, 16, T], F32R, kind="ExternalInput").ap()
    wqt_e = [nc.dram_tensor(f"wqt{j}", [128, 16, HD], F32R,
                            kind="ExternalInput").ap() for j in range(NR)]
    wkt_e = nc.dram_tensor("wkt", [128, 16, HD], F32R, kind="ExternalInput").ap()
    wvt_e = nc.dram_tensor("wvt", [128, 16, HD], F32R, kind="ExternalInput").ap()
    wot_e = nc.dram_tensor("wot", [128, NR, D], BF16, kind="ExternalInput").ap()
    cos_e = nc.dram_tensor("cosa", [128, T], F32R, kind="ExternalInput").ap()
    sin_e = nc.dram_tensor("sina", [128, T], F32R, kind="ExternalInput").ap()
    ident_e = nc.dram_tensor("ident", [128, 128], BF16, kind="ExternalInput").ap()
    ones_e = nc.dram_tensor("ones", [128, 128], BF16, kind="ExternalInput").ap()
    out_e = nc.dram_tensor("out", [T, D], BF16, kind="ExternalOutput").ap()

    with tile.TileContext(nc) as tc:
        import contextlib
        with contextlib.ExitStack() as ctx:
            consts = ctx.enter_context(tc.tile_pool(name="consts", bufs=1))
            weights = ctx.enter_context(tc.tile_pool(name="weights", bufs=1))
            acts = ctx.enter_context(tc.tile_pool(name="acts", bufs=1))

            cos_sb = consts.tile([128, T], F32R, tag="cos")
            sin_sb = consts.tile([128, T], F32R, tag="sin")
            ident_sb = consts.tile([128, 128], BF16, tag="ident")
            ones_sb = consts.tile([128, 128], BF16, tag="ones")
            wkt_sb = weights.tile([128, 16, HD], F32R, tag="wkt")
            wvt_sb = weights.tile([128, 16, HD], F32R, tag="wvt")
            wqt_sb = [weights.tile([128, 16, HD], F32R, tag=f"wqt{j}",
                                   name=f"wqt{j}_sb") for j in range(NR)]
            wot_sb = weights.tile([128, NR, D], BF16, tag="wot")
            # DMA ordering: weights ride the FRONT of both HWDGE queues (all
            # of them land by ~12us) so the chunk-paced projection matmuls
            # never stall on a weight; xt tile halves stripe across both
            # queues behind them in consumption order. No artificial gating
            # (the queues drain in emission order anyway); only wot (needed
            # ~100us in) rides SWDGE gated behind tile-0 k-proj.
            nc.sync.dma_start(out=wkt_sb[:, :8, :], in_=wkt_e[:, :8, :])
            nc.sync.dma_start(out=wkt_sb[:, 8:, :], in_=wkt_e[:, 8:, :])
            nc.sync.dma_start(out=wqt_sb[0][:, :8, :], in_=wqt_e[0][:, :8, :])
            nc.scalar.dma_start(out=wvt_sb, in_=wvt_e)
            nc.scalar.dma_start(out=wqt_sb[0][:, 8:, :], in_=wqt_e[0][:, 8:, :])
            nc.scalar.dma_start(out=wqt_sb[2], in_=wqt_e[2])
            nc.scalar.dma_start(out=wqt_sb[3], in_=wqt_e[3])

            qtr = [acts.tile([128, T], F32R, tag=f"qtr{j}", name=f"qtr{j}") for j in range(NR)]
            ktr = acts.tile([128, T], F32R, tag="ktr")
            v_sb = acts.tile([128, 16, HD], BF16, tag="vsb")  # v natural, s-chunked

            # ---------------- Phase 1: projections + RoPE + v transpose ----
            with tc.tile_pool(name="xt", bufs=8) as xt_pool, \
                 tc.tile_pool(name="rope", bufs=2) as rope_pool, \
                 tc.tile_pool(name="stage", bufs=3) as stage_pool, \
                 tc.tile_pool(name="p1ps", bufs=1, space="PSUM") as p1ps, \
                 tc.tile_pool(name="rotps", bufs=2, space="PSUM") as rotps:
                anchors = []  # tt0 k-proj matmul instructions
                for tt in range(4):
                    tsl = slice(tt * 512, (tt + 1) * 512)
                    xq = []
                    for i in range(4):
                        xti = xt_pool.tile([128, 4, 512], F32R, tag="xt")
                        lo = i * 4
                        nc.sync.dma_start(out=xti[:, 0:2, :],
                                          in_=xt_e[:, lo:lo + 2, tsl])
                        xq.append(xti)
                    if tt == 0:
                        # cos for t-tiles 0..1 behind xt0's sync half; sin
                        # rides scalar ahead of xt0[2:4] (rope k0 needs both
                        # by ~30us); wqt1 after xt0 on scalar — pass 2 uses
                        # q1 last
                        nc.sync.dma_start(out=cos_sb[:, :1024],
                                          in_=cos_e[:, :1024])
                        nc.scalar.dma_start(out=sin_sb[:, :1024],
                                            in_=sin_e[:, :1024])
                    for i in range(4):
                        lo = i * 4
                        nc.scalar.dma_start(out=xq[i][:, 2:4, :],
                                            in_=xt_e[:, lo + 2:lo + 4, tsl])
                    if tt == 0:
                        nc.scalar.dma_start(out=wqt_sb[1], in_=wqt_e[1])
                        nc.scalar.dma_start(out=ident_sb, in_=ident_e)
                        nc.scalar.dma_start(out=ones_sb, in_=ones_e)
                    if tt == 1:
                        # rope tables for t-tiles 2..3; behind xt1, ahead of
                        # xt2/xt3 on each queue
                        nc.sync.dma_start(out=cos_sb[:, 1024:],
                                          in_=cos_e[:, 1024:])
                        nc.scalar.dma_start(out=sin_sb[:, 1024:],
                                            in_=sin_e[:, 1024:])
                    qps = [p1ps.tile([128, 512], F32, tag=f"qps{j}", name=f"qps{j}_{tt}") for j in range(NR)]
                    kps = p1ps.tile([128, 512], F32, tag="kps")
                    vps = p1ps.tile([128, 512], F32, tag="vps")
                    # chunk-paced emission in DMA-arrival order: [0:2] halves
                    # of each i-group (sync queue), then [2:4] (scalar queue);
                    # per chunk the projections run back-to-back so the PE
                    # stream tracks delivery with no FIFO head-blocking. For
                    # tile 0 only wkt/wqt0/wvt/wqt2/wqt3 ride ahead of x, so
                    # q1..q3 go in a second pass (their weights land while
                    # pass 1 runs).
                    order = [(i, dc) for dc2 in range(2) for i in range(4)
                             for dc in (2 * dc2, 2 * dc2 + 1)]
                    passes = ([[("k",), ("q", 0), ("v",)], [("q", 2), ("q", 3), ("q", 1)]]
                              if tt == 0 else
                              [[("k",), ("q", 0), ("q", 1), ("q", 2), ("q", 3), ("v",)]])
                    for ops in passes:
                        for pos, (i, dc) in enumerate(order):
                            g = i * 4 + dc
                            first, last = pos == 0, pos == 15
                            if tt > 0 and pos < 2 and len(ops) == 6:
                                # previous tile's q ropes still drain the qps
                                # banks; lead with k/v so the PE FIFO isn't
                                # head-blocked; the skipped q's catch up at
                                # pos 2
                                ops_now = [("k",), ("v",)]
                            else:
                                ops_now = ops
                            if tt > 0 and pos == 2 and len(ops) == 6:
                                for qi, (ii, di) in enumerate(order[:2]):
                                    gi = ii * 4 + di
                                    for j in range(NR):
                                        nc.tensor.matmul(
                                            qps[j], wqt_sb[j][:, gi, :],
                                            xq[ii][:, di, :],
                                            start=(qi == 0), stop=False)
                            for op in ops_now:
                                if op[0] == "k":
                                    mk = nc.tensor.matmul(
                                        kps, wkt_sb[:, g, :], xq[i][:, dc, :],
                                        start=first, stop=last)
                                    if tt == 0:
                                        anchors.append(mk.ins)
                                elif op[0] == "v":
                                    nc.tensor.matmul(
                                        vps, wvt_sb[:, g, :], xq[i][:, dc, :],
                                        start=first, stop=last)
                                else:
                                    j = op[1]
                                    nc.tensor.matmul(
                                        qps[j], wqt_sb[j][:, g, :],
                                        xq[i][:, dc, :], start=first, stop=last)

                    # RoPE: dst = src*cos + rotate_half(src)*sin on DVE via
                    # partition-shifted PSUM reads (sign of the lower half
                    # folded into the host sin table). Mixed base partitions
                    # are only legal with a PSUM input, so the direct form
                    # must read PSUM — fine for tiles 0..2 where the banks
                    # recycle with slack.
                    def rope(src, dst):
                        t1 = rope_pool.tile([128, 512], F32R, tag="t1", name="t1")
                        nc.vector.tensor_mul(t1, src, cos_sb[:, tsl])
                        t2 = rope_pool.tile([128, 512], F32R, tag="t2", name="t2")
                        nc.vector.tensor_mul(t2[0:64, :], src[64:128, :],
                                             sin_sb[0:64, tsl])
                        nc.vector.tensor_mul(t2[64:128, :], src[0:64, :],
                                             sin_sb[64:128, tsl])
                        nc.gpsimd.tensor_add(dst, t1, t2)

                    # v: copy vT psum -> sbuf bf16, PE-transpose 128-blocks
                    vt_sb = rope_pool.tile([128, 512], BF16, tag="vt")
                    nc.scalar.copy(vt_sb, vps)
                    if tt < 3:
                        rope(kps, ktr[:, tsl])
                        for vb in range(4):
                            tr_ps = rotps.tile([128, 128], BF16, tag="rot")
                            nc.tensor.transpose(
                                tr_ps, vt_sb[:, vb * 128:(vb + 1) * 128],
                                ident_sb)
                            nc.vector.tensor_copy(v_sb[:, tt * 4 + vb, :], tr_ps)
                        for j in range(NR):
                            rope(qps[j], qtr[j][:, tsl])
                    else:
                        # Tile 3's ropes gate phase 2: the score PSUM tiles
                        # reuse these banks, so holding them through a ~9us
                        # DVE chain stalls the PE. Evacuate all five banks
                        # FIRST, split across the scalar+vector engines
                        # (~2.5us to free everything), then do the
                        # half-rotation with partition-shifted SBUF->SBUF
                        # DMAs (queues are idle by now) and base-aligned
                        # muls from SBUF.
                        # evacuate straight into the rope DESTINATION slice
                        # (scratch-free), rope in-place afterwards
                        srcs = [kps] + qps
                        dsts = [ktr[:, tsl]] + [qtr[j][:, tsl] for j in range(NR)]
                        for r, (src, dst) in enumerate(zip(srcs, dsts)):
                            if r % 2 == 0:
                                nc.scalar.copy(dst, src)
                            else:
                                nc.vector.tensor_copy(dst, src)
                        for vb in range(4):
                            tr_ps = rotps.tile([128, 128], BF16, tag="rot")
                            nc.tensor.transpose(
                                tr_ps, vt_sb[:, vb * 128:(vb + 1) * 128],
                                ident_sb)
                            nc.vector.tensor_copy(v_sb[:, tt * 4 + vb, :], tr_ps)
                        for stg, dst in zip(dsts, dsts):
                            stgr = stage_pool.tile([128, 512], F32R,
                                                   tag="rstgr", name="rstgr")
                            nc.sync.dma_start(out=stgr[0:64, :],
                                              in_=stg[64:128, :])
                            nc.scalar.dma_start(out=stgr[64:128, :],
                                                in_=stg[0:64, :])
                            t1 = rope_pool.tile([128, 512], F32R, tag="t1",
                                                name="t1")
                            nc.vector.tensor_mul(t1, stg, cos_sb[:, tsl])
                            t2 = rope_pool.tile([128, 512], F32R, tag="t2",
                                                name="t2")
                            nc.vector.tensor_mul(t2, stgr, sin_sb[:, tsl])
                            nc.gpsimd.tensor_add(dst, t1, t2)

            # ---------------- Phase 2+3: attention + out projection --------
            dwot = nc.gpsimd.dma_start(out=wot_sb, in_=wot_e)
            add_dep_helper(dwot.ins, anchors[15], reason="gate wot dma")
            with tc.tile_pool(name="p2sb", bufs=4) as p2sb, \
                 tc.tile_pool(name="dens", bufs=3) as dens, \
                 tc.tile_pool(name="otn", bufs=2) as otnp, \
                 tc.tile_pool(name="ostg", bufs=4) as ostg, \
                 tc.tile_pool(name="stps", bufs=2, space="PSUM") as stps, \
                 tc.tile_pool(name="otps", bufs=2, space="PSUM") as otps, \
                 tc.tile_pool(name="outps", bufs=2, space="PSUM") as outps:
                pending = [None]    # deferred softmax epilogue of previous head
                pend_out = []       # deferred out-projection pieces (prev tt)

                def flush_epilogue():
                    if pending[0] is not None:
                        pending[0]()
                        pending[0] = None

                def out_piece(tt, tkc, otn_t, dts):
                    # out-projection piece: 4 matmuls + evac + store per dt
                    rows = slice(tt * 512 + tkc * 128, tt * 512 + (tkc + 1) * 128)
                    for dt in dts:
                        o_ps = outps.tile([128, 512], F32, tag="ops",
                                          name=f"o_ps_{tt}_{tkc}_{dt}")
                        for hh in range(NR):
                            nc.tensor.matmul(
                                o_ps, otn_t[:, hh, tkc * 128:(tkc + 1) * 128],
                                wot_sb[:, hh, dt * 512:(dt + 1) * 512],
                                start=(hh == 0), stop=(hh == NR - 1))
                        o_sb = ostg.tile([128, 512], BF16, tag="ostg",
                                         name=f"o_sb_{tt}_{tkc}_{dt}")
                        nc.vector.tensor_copy(o_sb, o_ps)
                        nc.sync.dma_start(
                            out=out_e[rows, dt * 512:(dt + 1) * 512], in_=o_sb)

                for tt in range(4):
                    tsl = slice(tt * 512, (tt + 1) * 512)
                    otn_t = otnp.tile([128, NR, 512], BF16, tag="otn")
                    for h in range(NR):
                        ot_ps = otps.tile([128, 512], F32, tag="ot",
                                          name=f"ot_{tt}_{h}")
                        den = dens.tile([128, 2, 512], BF16, tag="den",
                                        name=f"den_{tt}_{h}")
                        exs = {}
                        # one-deep software pipeline: ST(sg) runs one step
                        # ahead of PV(sg) so PE never waits on the exp
                        for step in range(9):
                            if step < 8:
                                st_ps = stps.tile([128, 2, 512], F32, tag="st",
                                                  name=f"st_{tt}_{h}_{step}")
                                if tt == 0 and step < 6:
                                    # tile 0 has no out-proj filler yet, so
                                    # these heads run exp(ACT)-bound with PE
                                    # idle ~0.7us/step — enough for a HAM MID
                                    # window to re-throttle the clock. Keep
                                    # the PE warm with junk matmuls whose
                                    # target is wiped by the real ST's
                                    # start=True bank-clear.
                                    for _ in range(2):
                                        nc.tensor.matmul(
                                            st_ps[:, 0, 0:128], ident_sb,
                                            ident_sb, start=True, stop=True)
                                for half in range(2):
                                    sc = step * 2 + half
                                    nc.tensor.matmul(
                                        st_ps[:, half, :],
                                        ktr[:, sc * 128:(sc + 1) * 128],
                                        qtr[h][:, tsl], start=True, stop=True)
                                ex = p2sb.tile([128, 2, 512], BF16, tag="exp",
                                               name=f"ex_{tt}_{h}_{step}")
                                nc.scalar.activation(
                                    ex, st_ps, mybir.ActivationFunctionType.Exp,
                                    scale=SCALE)
                                exs[step] = ex
                                if step == 0:
                                    nc.vector.tensor_copy(den, ex)
                                else:
                                    nc.vector.tensor_add(den, den, ex)
                            if step >= 1:
                                sg = step - 1
                                for half in range(2):
                                    sc = sg * 2 + half
                                    nc.tensor.matmul(ot_ps, v_sb[:, sc, :],
                                                     exs[sg][:, half, :],
                                                     start=(sc == 0),
                                                     stop=(sc == 15))
                                if sg > 1:
                                    exs.pop(sg - 2, None)
                            if step == 2:
                                # previous head's epilogue: overlaps this
                                # head's score stream
                                flush_epilogue()
                            if step in (1, 5) and pend_out:
                                # a piece of the previous t-tile's out
                                # projection as PE filler; one right at step 1
                                # so the PE has work while exp(0) is in
                                # flight on ACT
                                pend_out.pop(0)()

                        def epilogue(ot_ps=ot_ps, den=den, h=h,
                                     otn_t=otn_t, tt=tt):
                            # partition-reduce+broadcast denominator on PE,
                            # both halves accumulated into one PSUM bank
                            bc_ps = outps.tile([128, 512], F32, tag="ops",
                                               name=f"bc_{tt}_{h}")
                            nc.tensor.matmul(bc_ps, ones_sb, den[:, 0, :],
                                             start=True, stop=False)
                            nc.tensor.matmul(bc_ps, ones_sb, den[:, 1, :],
                                             start=False, stop=True)
                            rden = dens.tile([128, 512], F32, tag="rden",
                                             name=f"rden_{tt}_{h}")
                            nc.vector.reciprocal_approx_fast(rden, bc_ps)
                            nc.vector.tensor_tensor(out=otn_t[:, h, :], in0=ot_ps,
                                                    in1=rden,
                                                    op=mybir.AluOpType.mult)
                        pending[0] = epilogue

                    flush_epilogue()
                    pend_out = [
                        (lambda tt=tt, tkc=tkc, otn_t=otn_t, dts=dts:
                         out_piece(tt, tkc, otn_t, dts))
                        for tkc in range(4) for dts in ((0, 1), (2, 3))]
                # final t-tile's out projection
                for p in pend_out:
                    p()
    nc.compile()
    return nc


def _get_nc():
    if "nc" not in _cache:
        _cache["nc"] = _build_nc()
    return _cache["nc"]


def _host_consts():
    if "consts" in _cache:
        return _cache["consts"]
    inv = 1.0 / (ROPE_BASE ** (np.arange(0, HD, 2, dtype=np.float64) / HD))
    freqs = np.outer(np.arange(T, dtype=np.float64), inv)  # [T, 64]
    emb = np.concatenate([freqs, freqs], axis=-1)  # [T, 128]
    cos_t = np.cos(emb).T.astype(np.float32).copy()  # [128, T]
    sin_t = np.sin(emb).T.astype(np.float32).copy()
    sin_t[:64, :] *= -1.0  # rotate-half sign folded in (see rope())
    ident = np.eye(128, dtype=ml_dtypes.bfloat16)
    ones = np.ones((128, 128), dtype=ml_dtypes.bfloat16)
    _cache["consts"] = (cos_t, sin_t, ident, ones)
    return _cache["consts"]


def _in_maps(x, wq, wk, wv, wo):
    cos_t, sin_t, ident, ones = _host_consts()
    maps = []
    for c in range(NCORES):
        b, g = c // KV, c % KV
        xt = np.ascontiguousarray(
            x[b].reshape(T, 16, 128).transpose(2, 1, 0)).astype(np.float32)
        wq_g = wq[g * NR * HD:(g + 1) * NR * HD]  # [512, D]
        # per-head contiguous slices: wqt{j}[p, dc, jc] = wq_g[j*128+jc, dc*128+p]
        wq_h = wq_g.reshape(NR, HD, 16, 128).transpose(0, 3, 2, 1)  # [j, p, dc, jc]
        wk_g = wk[g * HD:(g + 1) * HD]
        wkt = np.ascontiguousarray(wk_g.reshape(HD, 16, 128).transpose(2, 1, 0))
        wv_g = wv[g * HD:(g + 1) * HD]
        wvt = np.ascontiguousarray(wv_g.reshape(HD, 16, 128).transpose(2, 1, 0))
        wo_g = wo[:, g * NR * HD:(g + 1) * NR * HD]  # [D, 512]
        wot = np.ascontiguousarray(
            wo_g.reshape(D, NR, 128).transpose(2, 1, 0)).astype(ml_dtypes.bfloat16)
        m = {
            "xt": xt, "wkt": wkt.astype(np.float32),
            "wvt": wvt.astype(np.float32), "wot": wot,
            "cosa": cos_t, "sina": sin_t,
            "ident": ident, "ones": ones,
        }
        for j in range(NR):
            m[f"wqt{j}"] = np.ascontiguousarray(wq_h[j]).astype(np.float32)
        maps.append(m)
    return maps


def run_spmd(x, wq, wk, wv, wo, **kw):
    nc = _get_nc()
    maps = _in_maps(x, wq, wk, wv, wo)
    return run_bass_kernel_spmd(nc, maps, core_ids=list(range(NCORES)), **kw)


def kernel(x, wq, wk, wv, wo):
    res = run_spmd(x, wq, wk, wv, wo)
    out = np.zeros((B, T, D), dtype=np.float32)
    for c in range(NCORES):
        out[c // KV] += res.results[c]["out"].astype(np.float32)
    return out



# revision 39
# speedup vs baseline: 1.1662x; 1.0114x over previous
"""GQA attention kernel for 8 TRN2 NeuronCores.

Problem: B=2, T=2048, D=2048, H=16 q-heads, KV=4 kv-heads, HD=128, RoPE,
non-causal softmax, out projection. f32 reference.

Sharding: 8 cores = 2 batches x 4 kv-groups. Core c handles batch c//4 and
kv-group c%4 (4 q heads + 1 kv head). Each core computes a partial output
x @ wq_g -> attention -> (heads g) @ wo_g^T: full [T, D] partial summed on
host over the 4 groups of each batch (tensor-parallel unshard).

On-device layout: everything transposed ([hd, t], hd=128=partition dim).
 - host feeds xT, wqT, wkT, wvT (d-on-partition chunks) so projections are
   plain lhsT.T @ rhs matmuls with K=d contraction, fp32r (full PE rate).
 - scores computed transposed: ST[s, t] = k^T q per s-chunk; softmax over s
   (partitions) uses exp on ACT + bf16 chunk-adds on DVE + a ones-matmul
   partition-reduce-broadcast on PE; normalization folded into the OT evac.
 - PV: OT[hd, t] += v_nat[s, hd]^T expST[s, t] per s-chunk (bf16).
 - out projection: out[t, d] = sum_h OTn_h[j, t]^T wogT[j, d] (bf16).
"""
import os
import sys

for _p in ("/opt/trn_rl_repo", "/root/.axon_site/_ro/trn_rl_repo"):
    if os.path.isdir(_p) and _p not in sys.path:
        sys.path.append(_p)

import numpy as np
import ml_dtypes

import concourse.bass as bass
import concourse.tile as tile
from concourse.tile import add_dep_helper
from concourse import bacc, mybir
from concourse import bass_utils
from concourse.bass_utils import run_bass_kernel_spmd

# If a caller enables tracing (BASS_TRACE=1), artifact upload may have no
# bucket access in this container; fall back to the local dir.
_orig_upload = bass_utils.upload_artifacts


def _safe_upload(tmpdir):
    try:
        return _orig_upload(tmpdir)
    except Exception:
        return tmpdir


bass_utils.upload_artifacts = _safe_upload

B, T, D = 2, 2048, 2048
H, KV, HD = 16, 4, 128
NR = H // KV  # 4 q heads per kv group
NCORES = 8
ROPE_BASE = 10000.0
SCALE = float(HD) ** -0.5

F32R = mybir.dt.float32r
F32 = mybir.dt.float32
BF16 = mybir.dt.bfloat16

_cache = {}


def _build_nc():
    nc = bacc.Bacc("TRN2", target_bir_lowering=False, debug=False,
                   num_devices=NCORES)

    xt_e = nc.dram_tensor("xt", [128, 16, T], F32R, kind="ExternalInput").ap()
    wqt_e = [nc.dram_tensor(f"wqt{j}", [128, 16, HD], F32R,
                            kind="ExternalInput").ap() for j in range(NR)]
    wkt_e = nc.dram_tensor("wkt", [128, 16, HD], F32R, kind="ExternalInput").ap()
    wvt_e = nc.dram_tensor("wvt", [128, 16, HD], F32R, kind="ExternalInput").ap()
    wot_e = nc.dram_tensor("wot", [128, NR, D], BF16, kind="ExternalInput").ap()
    cos_e = nc.dram_tensor("cosa", [128, T], F32R, kind="ExternalInput").ap()
    sin_e = nc.dram_tensor("sina", [128, T], F32R, kind="ExternalInput").ap()
    ident_e = nc.dram_tensor("ident", [128, 128], BF16, kind="ExternalInput").ap()
    ones_e = nc.dram_tensor("ones", [128, 128], BF16, kind="ExternalInput").ap()
    out_e = nc.dram_tensor("out", [T, D], BF16, kind="ExternalOutput").ap()

    with tile.TileContext(nc) as tc:
        import contextlib
        with contextlib.ExitStack() as ctx:
            consts = ctx.enter_context(tc.tile_pool(name="consts", bufs=1))
            weights = ctx.enter_context(tc.tile_pool(name="weights", bufs=1))
            acts = ctx.enter_context(tc.tile_pool(name="acts", bufs=1))

            cos_sb = consts.tile([128, T], F32R, tag="cos")
            sin_sb = consts.tile([128, T], F32R, tag="sin")
            ident_sb = consts.tile([128, 128], BF16, tag="ident")
            ones_sb = consts.tile([128, 128], BF16, tag="ones")
            wkt_sb = weights.tile([128, 16, HD], F32R, tag="wkt")
            wvt_sb = weights.tile([128, 16, HD], F32R, tag="wvt")
            wqt_sb = [weights.tile([128, 16, HD], F32R, tag=f"wqt{j}",
                                   name=f"wqt{j}_sb") for j in range(NR)]
            wot_sb = weights.tile([128, NR, D], BF16, tag="wot")
            # DMA ordering: weights ride the FRONT of both HWDGE queues (all
            # of them land by ~12us) so the chunk-paced projection matmuls
            # never stall on a weight; xt tile halves stripe across both
            # queues behind them in consumption order. No artificial gating
            # (the queues drain in emission order anyway); only wot (needed
            # ~100us in) rides SWDGE gated behind tile-0 k-proj.
            nc.sync.dma_start(out=wkt_sb[:, :8, :], in_=wkt_e[:, :8, :])
            nc.scalar.dma_start(out=wvt_sb, in_=wvt_e)
            nc.scalar.dma_start(out=wqt_sb[0][:, 8:, :], in_=wqt_e[0][:, 8:, :])
            nc.scalar.dma_start(out=wqt_sb[2], in_=wqt_e[2])
            nc.scalar.dma_start(out=wqt_sb[3], in_=wqt_e[3])

            qtr = [acts.tile([128, T], F32R, tag=f"qtr{j}", name=f"qtr{j}") for j in range(NR)]
            ktr = acts.tile([128, T], F32R, tag="ktr")
            v_sb = acts.tile([128, 16, HD], BF16, tag="vsb")  # v natural, s-chunked

            # ---------------- Phase 1: projections + RoPE + v transpose ----
            with tc.tile_pool(name="xt", bufs=8) as xt_pool, \
                 tc.tile_pool(name="rope", bufs=2) as rope_pool, \
                 tc.tile_pool(name="stage", bufs=3) as stage_pool, \
                 tc.tile_pool(name="p1ps", bufs=1, space="PSUM") as p1ps, \
                 tc.tile_pool(name="rotps", bufs=2, space="PSUM") as rotps:
                anchors = []  # tt0 k-proj matmul instructions
                for tt in range(4):
                    tsl = slice(tt * 512, (tt + 1) * 512)
                    xq = []
                    for i in range(4):
                        xti = xt_pool.tile([128, 4, 512], F32R, tag="xt")
                        lo = i * 4
                        if tt == 0 and i == 0:
                            # split so the first k matmul only waits on
                            # wkt[:8] + 256KB; the rest of the startup
                            # weights ride behind it
                            nc.sync.dma_start(out=xti[:, 0:1, :],
                                              in_=xt_e[:, 0:1, tsl])
                            nc.sync.dma_start(out=wkt_sb[:, 8:, :],
                                              in_=wkt_e[:, 8:, :])
                            nc.sync.dma_start(out=wqt_sb[0][:, :8, :],
                                              in_=wqt_e[0][:, :8, :])
                            nc.sync.dma_start(out=xti[:, 1:2, :],
                                              in_=xt_e[:, 1:2, tsl])
                        else:
                            nc.sync.dma_start(out=xti[:, 0:2, :],
                                              in_=xt_e[:, lo:lo + 2, tsl])
                        xq.append(xti)
                    if tt == 0:
                        # q1 weights + cos tables for t-tiles 0..1 behind
                        # xt0's sync halves (pass 2 needs q1 ~40us in, rope
                        # k0 needs cos ~35us in); sin rides scalar ahead of
                        # xt0[2:4]
                        nc.sync.dma_start(out=wqt_sb[1], in_=wqt_e[1])
                        nc.sync.dma_start(out=cos_sb[:, :1024],
                                          in_=cos_e[:, :1024])
                        nc.scalar.dma_start(out=sin_sb[:, :1024],
                                            in_=sin_e[:, :1024])
                    for i in range(4):
                        lo = i * 4
                        nc.scalar.dma_start(out=xq[i][:, 2:4, :],
                                            in_=xt_e[:, lo + 2:lo + 4, tsl])
                    if tt == 0:
                        nc.scalar.dma_start(out=ident_sb, in_=ident_e)
                        nc.scalar.dma_start(out=ones_sb, in_=ones_e)
                    if tt == 1:
                        # rope tables for t-tiles 2..3; behind xt1, ahead of
                        # xt2/xt3 on each queue
                        nc.sync.dma_start(out=cos_sb[:, 1024:],
                                          in_=cos_e[:, 1024:])
                        nc.scalar.dma_start(out=sin_sb[:, 1024:],
                                            in_=sin_e[:, 1024:])
                    qps = [p1ps.tile([128, 512], F32, tag=f"qps{j}", name=f"qps{j}_{tt}") for j in range(NR)]
                    kps = p1ps.tile([128, 512], F32, tag="kps")
                    vps = p1ps.tile([128, 512], F32, tag="vps")
                    # chunk-paced emission in DMA-arrival order: [0:2] halves
                    # of each i-group (sync queue), then [2:4] (scalar queue);
                    # per chunk the projections run back-to-back so the PE
                    # stream tracks delivery with no FIFO head-blocking. For
                    # tile 0 only wkt/wqt0/wvt/wqt2/wqt3 ride ahead of x, so
                    # q1..q3 go in a second pass (their weights land while
                    # pass 1 runs).
                    order = [(i, dc) for dc2 in range(2) for i in range(4)
                             for dc in (2 * dc2, 2 * dc2 + 1)]
                    passes = ([[("k",), ("q", 0), ("v",)], [("q", 2), ("q", 3), ("q", 1)]]
                              if tt == 0 else
                              [[("k",), ("q", 0), ("q", 1), ("q", 2), ("q", 3), ("v",)]])
                    for ops in passes:
                        for pos, (i, dc) in enumerate(order):
                            g = i * 4 + dc
                            first, last = pos == 0, pos == 15
                            if tt > 0 and pos < 2 and len(ops) == 6:
                                # previous tile's q ropes still drain the qps
                                # banks; lead with k/v so the PE FIFO isn't
                                # head-blocked; the skipped q's catch up at
                                # pos 2
                                ops_now = [("k",), ("v",)]
                            else:
                                ops_now = ops
                            if tt > 0 and pos == 2 and len(ops) == 6:
                                for qi, (ii, di) in enumerate(order[:2]):
                                    gi = ii * 4 + di
                                    for j in range(NR):
                                        nc.tensor.matmul(
                                            qps[j], wqt_sb[j][:, gi, :],
                                            xq[ii][:, di, :],
                                            start=(qi == 0), stop=False)
                            for op in ops_now:
                                if op[0] == "k":
                                    mk = nc.tensor.matmul(
                                        kps, wkt_sb[:, g, :], xq[i][:, dc, :],
                                        start=first, stop=last)
                                    if tt == 0:
                                        anchors.append(mk.ins)
                                elif op[0] == "v":
                                    nc.tensor.matmul(
                                        vps, wvt_sb[:, g, :], xq[i][:, dc, :],
                                        start=first, stop=last)
                                else:
                                    j = op[1]
                                    nc.tensor.matmul(
                                        qps[j], wqt_sb[j][:, g, :],
                                        xq[i][:, dc, :], start=first, stop=last)

                    # RoPE: dst = src*cos + rotate_half(src)*sin on DVE via
                    # partition-shifted PSUM reads (sign of the lower half
                    # folded into the host sin table). Mixed base partitions
                    # are only legal with a PSUM input, so the direct form
                    # must read PSUM — fine for tiles 0..2 where the banks
                    # recycle with slack.
                    def rope(src, dst):
                        t1 = rope_pool.tile([128, 512], F32R, tag="t1", name="t1")
                        nc.vector.tensor_mul(t1, src, cos_sb[:, tsl])
                        t2 = rope_pool.tile([128, 512], F32R, tag="t2", name="t2")
                        nc.vector.tensor_mul(t2[0:64, :], src[64:128, :],
                                             sin_sb[0:64, tsl])
                        nc.vector.tensor_mul(t2[64:128, :], src[0:64, :],
                                             sin_sb[64:128, tsl])
                        nc.gpsimd.tensor_add(dst, t1, t2)

                    # v: copy vT psum -> sbuf bf16, PE-transpose 128-blocks
                    vt_sb = rope_pool.tile([128, 512], BF16, tag="vt")
                    nc.scalar.copy(vt_sb, vps)
                    if tt < 3:
                        rope(kps, ktr[:, tsl])
                        for vb in range(4):
                            tr_ps = rotps.tile([128, 128], BF16, tag="rot")
                            nc.tensor.transpose(
                                tr_ps, vt_sb[:, vb * 128:(vb + 1) * 128],
                                ident_sb)
                            nc.vector.tensor_copy(v_sb[:, tt * 4 + vb, :], tr_ps)
                        for j in range(NR):
                            rope(qps[j], qtr[j][:, tsl])
                    else:
                        # Tile 3's ropes gate phase 2: the score PSUM tiles
                        # reuse these banks, so holding them through a ~9us
                        # DVE chain stalls the PE. Evacuate all five banks
                        # FIRST, split across the scalar+vector engines
                        # (~2.5us to free everything), then do the
                        # half-rotation with partition-shifted SBUF->SBUF
                        # DMAs (queues are idle by now) and base-aligned
                        # muls from SBUF.
                        # evacuate straight into the rope DESTINATION slice
                        # (scratch-free), rope in-place afterwards
                        srcs = [kps] + qps
                        dsts = [ktr[:, tsl]] + [qtr[j][:, tsl] for j in range(NR)]
                        for r, (src, dst) in enumerate(zip(srcs, dsts)):
                            if r % 2 == 0:
                                nc.scalar.copy(dst, src)
                            else:
                                nc.vector.tensor_copy(dst, src)
                        for vb in range(4):
                            tr_ps = rotps.tile([128, 128], BF16, tag="rot")
                            nc.tensor.transpose(
                                tr_ps, vt_sb[:, vb * 128:(vb + 1) * 128],
                                ident_sb)
                            nc.vector.tensor_copy(v_sb[:, tt * 4 + vb, :], tr_ps)
                        for stg, dst in zip(dsts, dsts):
                            stgr = stage_pool.tile([128, 512], F32R,
                                                   tag="rstgr", name="rstgr")
                            nc.sync.dma_start(out=stgr[0:64, :],
                                              in_=stg[64:128, :])
                            nc.scalar.dma_start(out=stgr[64:128, :],
                                                in_=stg[0:64, :])
                            t1 = rope_pool.tile([128, 512], F32R, tag="t1",
                                                name="t1")
                            nc.vector.tensor_mul(t1, stg, cos_sb[:, tsl])
                            t2 = rope_pool.tile([128, 512], F32R, tag="t2",
                                                name="t2")
                            nc.vector.tensor_mul(t2, stgr, sin_sb[:, tsl])
                            nc.gpsimd.tensor_add(dst, t1, t2)

            # ---------------- Phase 2+3: attention + out projection --------
            dwot = nc.gpsimd.dma_start(out=wot_sb, in_=wot_e)
            add_dep_helper(dwot.ins, anchors[15], reason="gate wot dma")
            with tc.tile_pool(name="p2sb", bufs=4) as p2sb, \
                 tc.tile_pool(name="dens", bufs=3) as dens, \
                 tc.tile_pool(name="otn", bufs=2) as otnp, \
                 tc.tile_pool(name="ostg", bufs=4) as ostg, \
                 tc.tile_pool(name="stps", bufs=2, space="PSUM") as stps, \
                 tc.tile_pool(name="otps", bufs=2, space="PSUM") as otps, \
                 tc.tile_pool(name="outps", bufs=2, space="PSUM") as outps:
                pending = [None]    # deferred softmax epilogue of previous head
                pend_out = []       # deferred out-projection pieces (prev tt)

                def flush_epilogue():
                    if pending[0] is not None:
                        pending[0]()
                        pending[0] = None

                def out_piece(tt, tkc, otn_t, dts):
                    # out-projection piece: 4 matmuls + evac + store per dt
                    rows = slice(tt * 512 + tkc * 128, tt * 512 + (tkc + 1) * 128)
                    for dt in dts:
                        o_ps = outps.tile([128, 512], F32, tag="ops",
                                          name=f"o_ps_{tt}_{tkc}_{dt}")
                        for hh in range(NR):
                            nc.tensor.matmul(
                                o_ps, otn_t[:, hh, tkc * 128:(tkc + 1) * 128],
                                wot_sb[:, hh, dt * 512:(dt + 1) * 512],
                                start=(hh == 0), stop=(hh == NR - 1))
                        o_sb = ostg.tile([128, 512], BF16, tag="ostg",
                                         name=f"o_sb_{tt}_{tkc}_{dt}")
                        nc.vector.tensor_copy(o_sb, o_ps)
                        nc.sync.dma_start(
                            out=out_e[rows, dt * 512:(dt + 1) * 512], in_=o_sb)

                for tt in range(4):
                    tsl = slice(tt * 512, (tt + 1) * 512)
                    otn_t = otnp.tile([128, NR, 512], BF16, tag="otn")
                    for h in range(NR):
                        ot_ps = otps.tile([128, 512], F32, tag="ot",
                                          name=f"ot_{tt}_{h}")
                        den = dens.tile([128, 2, 512], BF16, tag="den",
                                        name=f"den_{tt}_{h}")
                        exs = {}
                        # one-deep software pipeline: ST(sg) runs one step
                        # ahead of PV(sg) so PE never waits on the exp
                        for step in range(9):
                            if step < 8:
                                st_ps = stps.tile([128, 2, 512], F32, tag="st",
                                                  name=f"st_{tt}_{h}_{step}")
                                if tt == 0 and step < 6:
                                    # tile 0 has no out-proj filler yet, so
                                    # these heads run exp(ACT)-bound with PE
                                    # idle ~0.7us/step — enough for a HAM MID
                                    # window to re-throttle the clock. Keep
                                    # the PE warm with junk matmuls whose
                                    # target is wiped by the real ST's
                                    # start=True bank-clear.
                                    for _ in range(2):
                                        nc.tensor.matmul(
                                            st_ps[:, 0, 0:128], ident_sb,
                                            ident_sb, start=True, stop=True)
                                for half in range(2):
                                    sc = step * 2 + half
                                    nc.tensor.matmul(
                                        st_ps[:, half, :],
                                        ktr[:, sc * 128:(sc + 1) * 128],
                                        qtr[h][:, tsl], start=True, stop=True)
                                ex = p2sb.tile([128, 2, 512], BF16, tag="exp",
                                               name=f"ex_{tt}_{h}_{step}")
                                nc.scalar.activation(
                                    ex, st_ps, mybir.ActivationFunctionType.Exp,
                                    scale=SCALE)
                                exs[step] = ex
                                if step == 0:
                                    nc.vector.tensor_copy(den, ex)
                                else:
                                    nc.vector.tensor_add(den, den, ex)
                            if step >= 1:
                                sg = step - 1
                                for half in range(2):
                                    sc = sg * 2 + half
                                    nc.tensor.matmul(ot_ps, v_sb[:, sc, :],
                                                     exs[sg][:, half, :],
                                                     start=(sc == 0),
                                                     stop=(sc == 15))
                                if sg > 1:
                                    exs.pop(sg - 2, None)
                            if step == 2:
                                # previous head's epilogue: overlaps this
                                # head's score stream
                                flush_epilogue()
                            if step in (1, 5) and pend_out:
                                # a piece of the previous t-tile's out
                                # projection as PE filler; one right at step 1
                                # so the PE has work while exp(0) is in
                                # flight on ACT
                                pend_out.pop(0)()

                        def epilogue(ot_ps=ot_ps, den=den, h=h,
                                     otn_t=otn_t, tt=tt):
                            # partition-reduce+broadcast denominator on PE,
                            # both halves accumulated into one PSUM bank
                            bc_ps = outps.tile([128, 512], F32, tag="ops",
                                               name=f"bc_{tt}_{h}")
                            nc.tensor.matmul(bc_ps, ones_sb, den[:, 0, :],
                                             start=True, stop=False)
                            nc.tensor.matmul(bc_ps, ones_sb, den[:, 1, :],
                                             start=False, stop=True)
                            rden = dens.tile([128, 512], F32, tag="rden",
                                             name=f"rden_{tt}_{h}")
                            nc.vector.reciprocal_approx_fast(rden, bc_ps)
                            nc.vector.tensor_tensor(out=otn_t[:, h, :], in0=ot_ps,
                                                    in1=rden,
                                                    op=mybir.AluOpType.mult)
                        pending[0] = epilogue

                    flush_epilogue()
                    pend_out = [
                        (lambda tt=tt, tkc=tkc, otn_t=otn_t, dts=dts:
                         out_piece(tt, tkc, otn_t, dts))
                        for tkc in range(4) for dts in ((0, 1), (2, 3))]
                # final t-tile's out projection
                for p in pend_out:
                    p()
    nc.compile()
    return nc


def _get_nc():
    if "nc" not in _cache:
        _cache["nc"] = _build_nc()
    return _cache["nc"]


def _host_consts():
    if "consts" in _cache:
        return _cache["consts"]
    inv = 1.0 / (ROPE_BASE ** (np.arange(0, HD, 2, dtype=np.float64) / HD))
    freqs = np.outer(np.arange(T, dtype=np.float64), inv)  # [T, 64]
    emb = np.concatenate([freqs, freqs], axis=-1)  # [T, 128]
    cos_t = np.cos(emb).T.astype(np.float32).copy()  # [128, T]
    sin_t = np.sin(emb).T.astype(np.float32).copy()
    sin_t[:64, :] *= -1.0  # rotate-half sign folded in (see rope())
    ident = np.eye(128, dtype=ml_dtypes.bfloat16)
    ones = np.ones((128, 128), dtype=ml_dtypes.bfloat16)
    _cache["consts"] = (cos_t, sin_t, ident, ones)
    return _cache["consts"]


def _in_maps(x, wq, wk, wv, wo):
    cos_t, sin_t, ident, ones = _host_consts()
    maps = []
    for c in range(NCORES):
        b, g = c // KV, c % KV
        xt = np.ascontiguousarray(
            x[b].reshape(T, 16, 128).transpose(2, 1, 0)).astype(np.float32)
        wq_g = wq[g * NR * HD:(g + 1) * NR * HD]  # [512, D]
        # per-head contiguous slices: wqt{j}[p, dc, jc] = wq_g[j*128+jc, dc*128+p]
        wq_h = wq_g.reshape(NR, HD, 16, 128).transpose(0, 3, 2, 1)  # [j, p, dc, jc]
        wk_g = wk[g * HD:(g + 1) * HD]
        wkt = np.ascontiguousarray(wk_g.reshape(HD, 16, 128).transpose(2, 1, 0))
        wv_g = wv[g * HD:(g + 1) * HD]
        wvt = np.ascontiguousarray(wv_g.reshape(HD, 16, 128).transpose(2, 1, 0))
        wo_g = wo[:, g * NR * HD:(g + 1) * NR * HD]  # [D, 512]
        wot = np.ascontiguousarray(
            wo_g.reshape(D, NR, 128).transpose(2, 1, 0)).astype(ml_dtypes.bfloat16)
        m = {
            "xt": xt, "wkt": wkt.astype(np.float32),
            "wvt": wvt.astype(np.float32), "wot": wot,
            "cosa": cos_t, "sina": sin_t,
            "ident": ident, "ones": ones,
        }
        for j in range(NR):
            m[f"wqt{j}"] = np.ascontiguousarray(wq_h[j]).astype(np.float32)
        maps.append(m)
    return maps


def run_spmd(x, wq, wk, wv, wo, **kw):
    nc = _get_nc()
    maps = _in_maps(x, wq, wk, wv, wo)
    return run_bass_kernel_spmd(nc, maps, core_ids=list(range(NCORES)), **kw)


def kernel(x, wq, wk, wv, wo):
    res = run_spmd(x, wq, wk, wv, wo)
    out = np.zeros((B, T, D), dtype=np.float32)
    for c in range(NCORES):
        out[c // KV] += res.results[c]["out"].astype(np.float32)
    return out



# revision 41
# speedup vs baseline: 1.1662x; 1.0000x over previous
"""GQA attention kernel for 8 TRN2 NeuronCores.

Problem: B=2, T=2048, D=2048, H=16 q-heads, KV=4 kv-heads, HD=128, RoPE,
non-causal softmax, out projection. f32 reference.

Sharding: 8 cores = 2 batches x 4 kv-groups. Core c handles batch c//4 and
kv-group c%4 (4 q heads + 1 kv head). Each core computes a partial output
x @ wq_g -> attention -> (heads g) @ wo_g^T: full [T, D] partial summed on
host over the 4 groups of each batch (tensor-parallel unshard).

On-device layout: everything transposed ([hd, t], hd=128=partition dim).
 - host feeds xT, wqT, wkT, wvT (d-on-partition chunks) so projections are
   plain lhsT.T @ rhs matmuls with K=d contraction, fp32r (full PE rate).
 - scores computed transposed: ST[s, t] = k^T q per s-chunk; softmax over s
   (partitions) uses exp on ACT + bf16 chunk-adds on DVE + a ones-matmul
   partition-reduce-broadcast on PE; normalization folded into the OT evac.
 - PV: OT[hd, t] += v_nat[s, hd]^T expST[s, t] per s-chunk (bf16).
 - out projection: out[t, d] = sum_h OTn_h[j, t]^T wogT[j, d] (bf16).
"""
import os
import sys

for _p in ("/opt/trn_rl_repo", "/root/.axon_site/_ro/trn_rl_repo"):
    if os.path.isdir(_p) and _p not in sys.path:
        sys.path.append(_p)

import numpy as np
import ml_dtypes

import concourse.bass as bass
import concourse.tile as tile
from concourse.tile import add_dep_helper
from concourse import bacc, mybir
from concourse import bass_utils
from concourse.bass_utils import run_bass_kernel_spmd

# If a caller enables tracing (BASS_TRACE=1), artifact upload may have no
# bucket access in this container; fall back to the local dir.
_orig_upload = bass_utils.upload_artifacts


def _safe_upload(tmpdir):
    try:
        return _orig_upload(tmpdir)
    except Exception:
        return tmpdir


bass_utils.upload_artifacts = _safe_upload

B, T, D = 2, 2048, 2048
H, KV, HD = 16, 4, 128
NR = H // KV  # 4 q heads per kv group
NCORES = 8
ROPE_BASE = 10000.0
SCALE = float(HD) ** -0.5

F32R = mybir.dt.float32r
F32 = mybir.dt.float32
BF16 = mybir.dt.bfloat16

_cache = {}


def _build_nc():
    nc = bacc.Bacc("TRN2", target_bir_lowering=False, debug=False,
                   num_devices=NCORES)

    xt_e = nc.dram_tensor("xt", [128, 16, T], F32R, kind="ExternalInput").ap()
    wqt_e = [nc.dram_tensor(f"wqt{j}", [128, 16, HD], F32R,
                            kind="ExternalInput").ap() for j in range(NR)]
    wkt_e = nc.dram_tensor("wkt", [128, 16, HD], F32R, kind="ExternalInput").ap()
    wvt_e = nc.dram_tensor("wvt", [128, 16, HD], F32R, kind="ExternalInput").ap()
    wot_e = nc.dram_tensor("wot", [128, NR, D], BF16, kind="ExternalInput").ap()
    cos_e = nc.dram_tensor("cosa", [128, T], F32R, kind="ExternalInput").ap()
    sin_e = nc.dram_tensor("sina", [128, T], F32R, kind="ExternalInput").ap()
    ident_e = nc.dram_tensor("ident", [128, 128], BF16, kind="ExternalInput").ap()
    ones_e = nc.dram_tensor("ones", [128, 128], BF16, kind="ExternalInput").ap()
    out_e = nc.dram_tensor("out", [T, D], BF16, kind="ExternalOutput").ap()

    with tile.TileContext(nc) as tc:
        import contextlib
        with contextlib.ExitStack() as ctx:
            consts = ctx.enter_context(tc.tile_pool(name="consts", bufs=1))
            weights = ctx.enter_context(tc.tile_pool(name="weights", bufs=1))
            acts = ctx.enter_context(tc.tile_pool(name="acts", bufs=1))

            cos_sb = consts.tile([128, T], F32R, tag="cos")
            sin_sb = consts.tile([128, T], F32R, tag="sin")
            ident_sb = consts.tile([128, 128], BF16, tag="ident")
            ones_sb = consts.tile([128, 128], BF16, tag="ones")
            wkt_sb = weights.tile([128, 16, HD], F32R, tag="wkt")
            wvt_sb = weights.tile([128, 16, HD], F32R, tag="wvt")
            wqt_sb = [weights.tile([128, 16, HD], F32R, tag=f"wqt{j}",
                                   name=f"wqt{j}_sb") for j in range(NR)]
            wot_sb = weights.tile([128, NR, D], BF16, tag="wot")
            # DMA ordering: weights ride the FRONT of both HWDGE queues (all
            # of them land by ~12us) so the chunk-paced projection matmuls
            # never stall on a weight; xt tile halves stripe across both
            # queues behind them in consumption order. No artificial gating
            # (the queues drain in emission order anyway); only wot (needed
            # ~100us in) rides SWDGE gated behind tile-0 k-proj.
            nc.sync.dma_start(out=wkt_sb[:, :8, :], in_=wkt_e[:, :8, :])
            nc.scalar.dma_start(out=wvt_sb, in_=wvt_e)
            nc.scalar.dma_start(out=wqt_sb[0][:, 8:, :], in_=wqt_e[0][:, 8:, :])
            nc.scalar.dma_start(out=wqt_sb[2], in_=wqt_e[2])
            nc.scalar.dma_start(out=wqt_sb[3], in_=wqt_e[3])

            qtr = [acts.tile([128, T], F32R, tag=f"qtr{j}", name=f"qtr{j}") for j in range(NR)]
            ktr = acts.tile([128, T], F32R, tag="ktr")
            v_sb = acts.tile([128, 16, HD], BF16, tag="vsb")  # v natural, s-chunked

            # ---------------- Phase 1: projections + RoPE + v transpose ----
            with tc.tile_pool(name="xt", bufs=8) as xt_pool, \
                 tc.tile_pool(name="rope", bufs=2) as rope_pool, \
                 tc.tile_pool(name="stage", bufs=3) as stage_pool, \
                 tc.tile_pool(name="p1ps", bufs=1, space="PSUM") as p1ps, \
                 tc.tile_pool(name="rotps", bufs=2, space="PSUM") as rotps:
                anchors = []  # tt0 k-proj matmul instructions
                for tt in range(4):
                    tsl = slice(tt * 512, (tt + 1) * 512)
                    xq = []
                    for i in range(4):
                        xti = xt_pool.tile([128, 4, 512], F32R, tag="xt")
                        lo = i * 4
                        if tt == 0 and i == 0:
                            # split so the first k matmul only waits on
                            # wkt[:8] + 256KB; the rest of the startup
                            # weights ride behind it
                            nc.sync.dma_start(out=xti[:, 0:1, :],
                                              in_=xt_e[:, 0:1, tsl])
                            nc.sync.dma_start(out=wkt_sb[:, 8:, :],
                                              in_=wkt_e[:, 8:, :])
                            nc.sync.dma_start(out=wqt_sb[0][:, :8, :],
                                              in_=wqt_e[0][:, :8, :])
                            nc.sync.dma_start(out=xti[:, 1:2, :],
                                              in_=xt_e[:, 1:2, tsl])
                        else:
                            nc.sync.dma_start(out=xti[:, 0:2, :],
                                              in_=xt_e[:, lo:lo + 2, tsl])
                        xq.append(xti)
                    if tt == 0:
                        # q1 weights + cos tables for t-tiles 0..1 behind
                        # xt0's sync halves (pass 2 needs q1 ~40us in, rope
                        # k0 needs cos ~35us in); sin rides scalar ahead of
                        # xt0[2:4]
                        nc.sync.dma_start(out=wqt_sb[1], in_=wqt_e[1])
                        nc.sync.dma_start(out=cos_sb[:, :1024],
                                          in_=cos_e[:, :1024])
                        nc.scalar.dma_start(out=sin_sb[:, :1024],
                                            in_=sin_e[:, :1024])
                    for i in range(4):
                        lo = i * 4
                        nc.scalar.dma_start(out=xq[i][:, 2:4, :],
                                            in_=xt_e[:, lo + 2:lo + 4, tsl])
                    if tt == 0:
                        nc.scalar.dma_start(out=ident_sb, in_=ident_e)
                        nc.scalar.dma_start(out=ones_sb, in_=ones_e)
                    if tt == 1:
                        # rope tables for t-tiles 2..3; behind xt1, ahead of
                        # xt2/xt3 on each queue
                        nc.sync.dma_start(out=cos_sb[:, 1024:],
                                          in_=cos_e[:, 1024:])
                        nc.scalar.dma_start(out=sin_sb[:, 1024:],
                                            in_=sin_e[:, 1024:])
                    qps = [p1ps.tile([128, 512], F32, tag=f"qps{j}", name=f"qps{j}_{tt}") for j in range(NR)]
                    kps = p1ps.tile([128, 512], F32, tag="kps")
                    vps = p1ps.tile([128, 512], F32, tag="vps")
                    # chunk-paced emission in DMA-arrival order: [0:2] halves
                    # of each i-group (sync queue), then [2:4] (scalar queue);
                    # per chunk the projections run back-to-back so the PE
                    # stream tracks delivery with no FIFO head-blocking. For
                    # tile 0 only wkt/wqt0/wvt/wqt2/wqt3 ride ahead of x, so
                    # q1..q3 go in a second pass (their weights land while
                    # pass 1 runs).
                    order = [(i, dc) for dc2 in range(2) for i in range(4)
                             for dc in (2 * dc2, 2 * dc2 + 1)]
                    passes = ([[("k",), ("q", 0), ("v",)], [("q", 2), ("q", 3), ("q", 1)]]
                              if tt == 0 else
                              [[("k",), ("q", 0), ("q", 1), ("q", 2), ("q", 3), ("v",)]])
                    for ops in passes:
                        for pos, (i, dc) in enumerate(order):
                            g = i * 4 + dc
                            first, last = pos == 0, pos == 15
                            if tt > 0 and pos < 2 and len(ops) == 6:
                                # previous tile's q ropes still drain the qps
                                # banks; lead with v/k (vps frees fastest via
                                # the vt copy) so the PE FIFO isn't
                                # head-blocked; the skipped q's catch up at
                                # pos 2
                                ops_now = [("v",), ("k",)]
                            else:
                                ops_now = ops
                            if tt > 0 and pos == 2 and len(ops) == 6:
                                for qi, (ii, di) in enumerate(order[:2]):
                                    gi = ii * 4 + di
                                    for j in range(NR):
                                        nc.tensor.matmul(
                                            qps[j], wqt_sb[j][:, gi, :],
                                            xq[ii][:, di, :],
                                            start=(qi == 0), stop=False)
                            for op in ops_now:
                                if op[0] == "k":
                                    mk = nc.tensor.matmul(
                                        kps, wkt_sb[:, g, :], xq[i][:, dc, :],
                                        start=first, stop=last)
                                    if tt == 0:
                                        anchors.append(mk.ins)
                                elif op[0] == "v":
                                    nc.tensor.matmul(
                                        vps, wvt_sb[:, g, :], xq[i][:, dc, :],
                                        start=first, stop=last)
                                else:
                                    j = op[1]
                                    nc.tensor.matmul(
                                        qps[j], wqt_sb[j][:, g, :],
                                        xq[i][:, dc, :], start=first, stop=last)

                    # RoPE: dst = src*cos + rotate_half(src)*sin on DVE via
                    # partition-shifted PSUM reads (sign of the lower half
                    # folded into the host sin table). Mixed base partitions
                    # are only legal with a PSUM input, so the direct form
                    # must read PSUM — fine for tiles 0..2 where the banks
                    # recycle with slack.
                    def rope(src, dst):
                        t1 = rope_pool.tile([128, 512], F32R, tag="t1", name="t1")
                        nc.vector.tensor_mul(t1, src, cos_sb[:, tsl])
                        t2 = rope_pool.tile([128, 512], F32R, tag="t2", name="t2")
                        nc.vector.tensor_mul(t2[0:64, :], src[64:128, :],
                                             sin_sb[0:64, tsl])
                        nc.vector.tensor_mul(t2[64:128, :], src[0:64, :],
                                             sin_sb[64:128, tsl])
                        nc.gpsimd.tensor_add(dst, t1, t2)

                    # v: copy vT psum -> sbuf bf16, PE-transpose 128-blocks
                    vt_sb = rope_pool.tile([128, 512], BF16, tag="vt")
                    nc.scalar.copy(vt_sb, vps)
                    if tt < 3:
                        rope(kps, ktr[:, tsl])
                        for vb in range(4):
                            tr_ps = rotps.tile([128, 128], BF16, tag="rot")
                            nc.tensor.transpose(
                                tr_ps, vt_sb[:, vb * 128:(vb + 1) * 128],
                                ident_sb)
                            nc.vector.tensor_copy(v_sb[:, tt * 4 + vb, :], tr_ps)
                        for j in range(NR):
                            rope(qps[j], qtr[j][:, tsl])
                    else:
                        # Tile 3's ropes gate phase 2: the score PSUM tiles
                        # reuse these banks, so holding them through a ~9us
                        # DVE chain stalls the PE. Evacuate all five banks
                        # FIRST, split across the scalar+vector engines
                        # (~2.5us to free everything), then do the
                        # half-rotation with partition-shifted SBUF->SBUF
                        # DMAs (queues are idle by now) and base-aligned
                        # muls from SBUF.
                        # evacuate straight into the rope DESTINATION slice
                        # (scratch-free), rope in-place afterwards
                        srcs = [kps] + qps
                        dsts = [ktr[:, tsl]] + [qtr[j][:, tsl] for j in range(NR)]
                        for r, (src, dst) in enumerate(zip(srcs, dsts)):
                            if r % 2 == 0:
                                nc.scalar.copy(dst, src)
                            else:
                                nc.vector.tensor_copy(dst, src)
                        for vb in range(4):
                            tr_ps = rotps.tile([128, 128], BF16, tag="rot")
                            nc.tensor.transpose(
                                tr_ps, vt_sb[:, vb * 128:(vb + 1) * 128],
                                ident_sb)
                            nc.vector.tensor_copy(v_sb[:, tt * 4 + vb, :], tr_ps)
                        # bridge the PE across the evac/first-exp window so a
                        # HAM MID window can't re-throttle the clock right as
                        # phase 2 starts; outputs are never read
                        for w in range(8):
                            tr_ps = rotps.tile([128, 128], BF16, tag="rot",
                                               name=f"warm{w}")
                            nc.tensor.transpose(tr_ps, ident_sb, ident_sb)
                        for stg, dst in zip(dsts, dsts):
                            stgr = stage_pool.tile([128, 512], F32R,
                                                   tag="rstgr", name="rstgr")
                            nc.sync.dma_start(out=stgr[0:64, :],
                                              in_=stg[64:128, :])
                            nc.scalar.dma_start(out=stgr[64:128, :],
                                                in_=stg[0:64, :])
                            t1 = rope_pool.tile([128, 512], F32R, tag="t1",
                                                name="t1")
                            nc.vector.tensor_mul(t1, stg, cos_sb[:, tsl])
                            t2 = rope_pool.tile([128, 512], F32R, tag="t2",
                                                name="t2")
                            nc.vector.tensor_mul(t2, stgr, sin_sb[:, tsl])
                            nc.gpsimd.tensor_add(dst, t1, t2)

            # ---------------- Phase 2+3: attention + out projection --------
            dwot = nc.gpsimd.dma_start(out=wot_sb, in_=wot_e)
            add_dep_helper(dwot.ins, anchors[15], reason="gate wot dma")
            with tc.tile_pool(name="p2sb", bufs=4) as p2sb, \
                 tc.tile_pool(name="dens", bufs=3) as dens, \
                 tc.tile_pool(name="otn", bufs=2) as otnp, \
                 tc.tile_pool(name="ostg", bufs=4) as ostg, \
                 tc.tile_pool(name="stps", bufs=2, space="PSUM") as stps, \
                 tc.tile_pool(name="otps", bufs=2, space="PSUM") as otps, \
                 tc.tile_pool(name="outps", bufs=2, space="PSUM") as outps:
                pending = [None]    # deferred softmax epilogue of previous head
                pend_out = []       # deferred out-projection pieces (prev tt)

                def flush_epilogue():
                    if pending[0] is not None:
                        pending[0]()
                        pending[0] = None

                def out_piece(tt, tkc, otn_t, dts):
                    # out-projection piece: 4 matmuls + evac + store per dt
                    rows = slice(tt * 512 + tkc * 128, tt * 512 + (tkc + 1) * 128)
                    for dt in dts:
                        o_ps = outps.tile([128, 512], F32, tag="ops",
                                          name=f"o_ps_{tt}_{tkc}_{dt}")
                        for hh in range(NR):
                            nc.tensor.matmul(
                                o_ps, otn_t[:, hh, tkc * 128:(tkc + 1) * 128],
                                wot_sb[:, hh, dt * 512:(dt + 1) * 512],
                                start=(hh == 0), stop=(hh == NR - 1))
                        o_sb = ostg.tile([128, 512], BF16, tag="ostg",
                                         name=f"o_sb_{tt}_{tkc}_{dt}")
                        nc.vector.tensor_copy(o_sb, o_ps)
                        nc.sync.dma_start(
                            out=out_e[rows, dt * 512:(dt + 1) * 512], in_=o_sb)

                for tt in range(4):
                    tsl = slice(tt * 512, (tt + 1) * 512)
                    otn_t = otnp.tile([128, NR, 512], BF16, tag="otn")
                    for h in range(NR):
                        ot_ps = otps.tile([128, 512], F32, tag="ot",
                                          name=f"ot_{tt}_{h}")
                        den = dens.tile([128, 2, 512], BF16, tag="den",
                                        name=f"den_{tt}_{h}")
                        exs = {}
                        # one-deep software pipeline: ST(sg) runs one step
                        # ahead of PV(sg) so PE never waits on the exp
                        for step in range(9):
                            if step < 8:
                                st_ps = stps.tile([128, 2, 512], F32, tag="st",
                                                  name=f"st_{tt}_{h}_{step}")
                                if tt == 0 and step < 6:
                                    # tile 0 has no out-proj filler yet, so
                                    # these heads run exp(ACT)-bound with PE
                                    # idle ~0.7us/step — enough for a HAM MID
                                    # window to re-throttle the clock. Keep
                                    # the PE warm with junk matmuls whose
                                    # target is wiped by the real ST's
                                    # start=True bank-clear.
                                    for _ in range(2):
                                        nc.tensor.matmul(
                                            st_ps[:, 0, 0:128], ident_sb,
                                            ident_sb, start=True, stop=True)
                                for half in range(2):
                                    sc = step * 2 + half
                                    nc.tensor.matmul(
                                        st_ps[:, half, :],
                                        ktr[:, sc * 128:(sc + 1) * 128],
                                        qtr[h][:, tsl], start=True, stop=True)
                                ex = p2sb.tile([128, 2, 512], BF16, tag="exp",
                                               name=f"ex_{tt}_{h}_{step}")
                                nc.scalar.activation(
                                    ex, st_ps, mybir.ActivationFunctionType.Exp,
                                    scale=SCALE)
                                exs[step] = ex
                                if step == 0:
                                    nc.vector.tensor_copy(den, ex)
                                else:
                                    nc.vector.tensor_add(den, den, ex)
                            if step >= 1:
                                sg = step - 1
                                for half in range(2):
                                    sc = sg * 2 + half
                                    nc.tensor.matmul(ot_ps, v_sb[:, sc, :],
                                                     exs[sg][:, half, :],
                                                     start=(sc == 0),
                                                     stop=(sc == 15))
                                if sg > 1:
                                    exs.pop(sg - 2, None)
                            if step == 2:
                                # previous head's epilogue: overlaps this
                                # head's score stream
                                flush_epilogue()
                            if step in (1, 5) and pend_out:
                                # a piece of the previous t-tile's out
                                # projection as PE filler; one right at step 1
                                # so the PE has work while exp(0) is in
                                # flight on ACT
                                pend_out.pop(0)()

                        def epilogue(ot_ps=ot_ps, den=den, h=h,
                                     otn_t=otn_t, tt=tt):
                            # partition-reduce+broadcast denominator on PE,
                            # both halves accumulated into one PSUM bank
                            bc_ps = outps.tile([128, 512], F32, tag="ops",
                                               name=f"bc_{tt}_{h}")
                            nc.tensor.matmul(bc_ps, ones_sb, den[:, 0, :],
                                             start=True, stop=False)
                            nc.tensor.matmul(bc_ps, ones_sb, den[:, 1, :],
                                             start=False, stop=True)
                            rden = dens.tile([128, 512], F32, tag="rden",
                                             name=f"rden_{tt}_{h}")
                            nc.vector.reciprocal_approx_fast(rden, bc_ps)
                            nc.vector.tensor_tensor(out=otn_t[:, h, :], in0=ot_ps,
                                                    in1=rden,
                                                    op=mybir.AluOpType.mult)
                        pending[0] = epilogue

                    flush_epilogue()
                    pend_out = [
                        (lambda tt=tt, tkc=tkc, otn_t=otn_t, dts=dts:
                         out_piece(tt, tkc, otn_t, dts))
                        for tkc in range(4) for dts in ((0, 1), (2, 3))]
                # final t-tile's out projection
                for p in pend_out:
                    p()
    nc.compile()
    return nc


def _get_nc():
    if "nc" not in _cache:
        _cache["nc"] = _build_nc()
    return _cache["nc"]


def _host_consts():
    if "consts" in _cache:
        return _cache["consts"]
    inv = 1.0 / (ROPE_BASE ** (np.arange(0, HD, 2, dtype=np.float64) / HD))
    freqs = np.outer(np.arange(T, dtype=np.float64), inv)  # [T, 64]
    emb = np.concatenate([freqs, freqs], axis=-1)  # [T, 128]
    cos_t = np.cos(emb).T.astype(np.float32).copy()  # [128, T]
    sin_t = np.sin(emb).T.astype(np.float32).copy()
    sin_t[:64, :] *= -1.0  # rotate-half sign folded in (see rope())
    ident = np.eye(128, dtype=ml_dtypes.bfloat16)
    ones = np.ones((128, 128), dtype=ml_dtypes.bfloat16)
    _cache["consts"] = (cos_t, sin_t, ident, ones)
    return _cache["consts"]


def _in_maps(x, wq, wk, wv, wo):
    cos_t, sin_t, ident, ones = _host_consts()
    maps = []
    for c in range(NCORES):
        b, g = c // KV, c % KV
        xt = np.ascontiguousarray(
            x[b].reshape(T, 16, 128).transpose(2, 1, 0)).astype(np.float32)
        wq_g = wq[g * NR * HD:(g + 1) * NR * HD]  # [512, D]
        # per-head contiguous slices: wqt{j}[p, dc, jc] = wq_g[j*128+jc, dc*128+p]
        wq_h = wq_g.reshape(NR, HD, 16, 128).transpose(0, 3, 2, 1)  # [j, p, dc, jc]
        wk_g = wk[g * HD:(g + 1) * HD]
        wkt = np.ascontiguousarray(wk_g.reshape(HD, 16, 128).transpose(2, 1, 0))
        wv_g = wv[g * HD:(g + 1) * HD]
        wvt = np.ascontiguousarray(wv_g.reshape(HD, 16, 128).transpose(2, 1, 0))
        wo_g = wo[:, g * NR * HD:(g + 1) * NR * HD]  # [D, 512]
        wot = np.ascontiguousarray(
            wo_g.reshape(D, NR, 128).transpose(2, 1, 0)).astype(ml_dtypes.bfloat16)
        m = {
            "xt": xt, "wkt": wkt.astype(np.float32),
            "wvt": wvt.astype(np.float32), "wot": wot,
            "cosa": cos_t, "sina": sin_t,
            "ident": ident, "ones": ones,
        }
        for j in range(NR):
            m[f"wqt{j}"] = np.ascontiguousarray(wq_h[j]).astype(np.float32)
        maps.append(m)
    return maps


def run_spmd(x, wq, wk, wv, wo, **kw):
    nc = _get_nc()
    maps = _in_maps(x, wq, wk, wv, wo)
    return run_bass_kernel_spmd(nc, maps, core_ids=list(range(NCORES)), **kw)


def kernel(x, wq, wk, wv, wo):
    res = run_spmd(x, wq, wk, wv, wo)
    out = np.zeros((B, T, D), dtype=np.float32)
    for c in range(NCORES):
        out[c // KV] += res.results[c]["out"].astype(np.float32)
    return out

